# revision 2
# baseline (speedup 1.0000x reference)
"""DownsampleExtractor Trainium2 kernel.

Math refactoring (exact up to fp reassociation):
  The reference projects K and V per group (B*L*T rows x 1152 -> 512) and then
  does NQ=1 cross-attention. With a single query per layer this collapses:

  scores[b,l,h,t] = Qp[l,h,:] . Kp[b,l,t,h,:]           (Kp = K @ Wk + bk)
                  = K[b,l,t,:] . (Wk[g] @ Qp_head) + const(l,h)
  The const is invariant over t -> dropped (softmax shift invariance).
  So scores = K[b,l] @ wtil[l]   with wtil[l] = SCALE * Wk[g] @ Qp heads, (1152 x 8).

  pooled[b,l,h,e] = sum_t attn[t] * Vp[t, h*64+e]
                  = (sum_t attn[h,t] V[b,l,t,:]) @ Wv[g][:, h*64+e] + bv  (attn sums to 1)
  So attention is applied to RAW V (A = attn @ V, 8 x 1152), then projected per head.
  This avoids the 130 GFLOP K/V projections entirely (~2.8 GFLOP total).

  The head_dim-major flatten (f = e*8+h) before Wo is handled by row-permuting
  Wo on the host (Wo_p[h*64+e] = Wo[e*8+h]) so the device uses h-major layout.
  bv's contribution is folded into the output bias: bo' = bo + bv @ Wo_p.

Sharding: 72 (b, g) group-instances over 8 cores: core c owns group c for all
8 b (24 layer-instances) plus group 8 for b=c (3 layer-instances). Each core
streams only its own K/V (64 MB) and 2 groups of weights.

All device DMA loads are contiguous-per-partition; the host pre-transposes
K to (d, t) layout (free) and packs K^T and V per layer-instance into one
2.36 MB block so each instance is a single large efficient DMA.
"""

import math

import numpy as np

# hardcoded problem dims
B, L, T, D = 8, 27, 256, 1152
GS = 3
G = L // GS
DD = 512
H, HD = 8, 64
OD = 2048
SCALE = 1.0 / math.sqrt(HD)
NCORES = 8
DB = D // 128   # 9 contraction blocks
TB = T // 128   # 2 token blocks
PB = DD // 128  # 4 blocks of the 512-dim pooled vector
NI = 27         # layer-instances per core (24 main group + 3 aux group)
NMAIN = 24

_NC_CACHE = None


def _build_bass():
    import concourse.bacc as bacc
    import concourse.tile as tile
    import concourse.mybir as mybir
    from concourse.masks import make_identity

    f32 = mybir.dt.float32
    nc = bacc.Bacc(None, target_bir_lowering=False)

    kv = nc.dram_tensor("kv", (NI, 128, 2 * 2304), f32, kind="ExternalInput")
    wt = nc.dram_tensor("wt", (128, NI, DB, H), f32, kind="ExternalInput")
    wv = nc.dram_tensor("wv", (2, 128, DB, DD), f32, kind="ExternalInput")
    wo = nc.dram_tensor("wo", (2, 128, PB, OD), f32, kind="ExternalInput")
    bo = nc.dram_tensor("bo", (2, OD), f32, kind="ExternalInput")
    out = nc.dram_tensor("out", (NI, OD), f32, kind="ExternalOutput")

    with tile.TileContext(nc) as tc:
        with (
            tc.tile_pool(name="const", bufs=1) as const,
            tc.tile_pool(name="kvp", bufs=3) as kvp,
            tc.tile_pool(name="wvp", bufs=1) as wvp,
            tc.tile_pool(name="wop", bufs=1) as wop,
            tc.tile_pool(name="bop", bufs=1) as bop,
            tc.tile_pool(name="atp", bufs=2) as atp,
            tc.tile_pool(name="sm", bufs=3) as sm,
            tc.tile_pool(name="grp", bufs=2) as grp,
            tc.tile_pool(name="ps_sc", bufs=2, space="PSUM") as ps_sc,
            tc.tile_pool(name="ps_tr", bufs=1, space="PSUM") as ps_tr,
            tc.tile_pool(name="ps_at", bufs=2, space="PSUM") as ps_at,
            tc.tile_pool(name="ps_pool", bufs=1, space="PSUM") as ps_pool,
            tc.tile_pool(name="ps_fin", bufs=2, space="PSUM") as ps_fin,
        ):
            ident = const.tile([128, 128], f32)
            make_identity(nc, ident)
            ones = const.tile([1, NMAIN], f32)
            nc.vector.memset(ones, 1.0)

            wt_sb = const.tile([128, NI, DB, H], f32)
            nc.sync.dma_start(out=wt_sb, in_=wt[:, :, :, :])

            def instance(i, at_sb, icol):
                kvt = kvp.tile([128, 2 * 2304], f32)
                nc.sync.dma_start(out=kvt, in_=kv[i, :, :])
                kt = kvt[:, :2304].rearrange("p (db t) -> p db t", db=DB)
                vt = kvt[:, 2304:].rearrange("p (tb d) -> p tb d", tb=TB)

                # scores^T (h x t) = sum_db wtil_block^T.T @ K^T_block
                sc = ps_sc.tile([H, T], f32)
                for db in range(DB):
                    nc.tensor.matmul(
                        sc,
                        wt_sb[:, i, db, :],
                        kt[:, db, :],
                        start=(db == 0),
                        stop=(db == DB - 1),
                    )
                # softmax over t (free dim); logits are O(1) so no max shift
                exps = sm.tile([H, T], f32)
                sums = sm.tile([H, 1], f32)
                nc.scalar.activation(
                    out=exps, in_=sc,
                    func=mybir.ActivationFunctionType.Exp,
                    accum_out=sums,
                )
                rec = sm.tile([H, 1], f32)
                nc.vector.reciprocal(rec, sums)
                attn = sm.tile([H, T], f32)
                nc.vector.tensor_scalar_mul(out=attn, in0=exps, scalar1=rec)

                # attn^T via PE transpose: (8 x 128)->(128 x 8) per t-block
                atr_ps = ps_tr.tile([128, TB, H], f32)
                for tb in range(TB):
                    nc.tensor.transpose(
                        atr_ps[:, tb, :],
                        attn[:, tb * 128:(tb + 1) * 128],
                        ident[:H, :H],
                    )
                attnT = sm.tile([128, TB, H], f32)
                nc.vector.tensor_copy(out=attnT, in_=atr_ps)

                # A^T blocks: (128d x 8h) = V_block(t x d).T @ attn^T(t x h)
                at_ps = ps_at.tile([128, DB, H], f32)
                for db in range(DB):
                    for tb in range(TB):
                        nc.tensor.matmul(
                            at_ps[:, db, :],
                            vt[:, tb, db * 128:(db + 1) * 128],
                            attnT[:, tb, :],
                            start=(tb == 0),
                            stop=(tb == TB - 1),
                        )
                nc.vector.tensor_copy(out=at_sb[:, :, :, icol], in_=at_ps)

            def group_tail(gi, at_sb, ninst, row0):
                wv_sb = wvp.tile([128, DB, DD], f32)
                nc.sync.dma_start(out=wv_sb, in_=wv[gi, :, :, :])
                wo_sb = wop.tile([128, PB, OD], f32)
                nc.sync.dma_start(out=wo_sb, in_=wo[gi, :, :, :])
                bo_sb = bop.tile([1, OD], f32)
                nc.sync.dma_start(out=bo_sb, in_=bo[gi:gi + 1, :])

                # pooled'^T: per head h: (64e x ninst) = Wv_hslice.T @ A^T cols
                pfT = grp.tile([128, PB, NMAIN], f32)
                for h in range(H):
                    pl = ps_pool.tile([64, NMAIN], f32)
                    for db in range(DB):
                        nc.tensor.matmul(
                            pl[:, :ninst],
                            wv_sb[:, db, h * 64:(h + 1) * 64],
                            at_sb[:, db, h, :ninst],
                            start=(db == 0),
                            stop=(db == DB - 1),
                        )
                    nc.vector.tensor_copy(
                        out=pfT[(h % 2) * 64:(h % 2) * 64 + 64, h // 2, :ninst],
                        in_=pl[:, :ninst],
                    )

                # out rows = bo' + sum_pb pfT_block.T @ Wo'_block
                osb = grp.tile([NMAIN, OD], f32)
                for oc in range(OD // 512):
                    fin = ps_fin.tile([NMAIN, 512], f32)
                    nc.tensor.matmul(
                        fin[:ninst, :],
                        ones[:, :ninst],
                        bo_sb[:, oc * 512:(oc + 1) * 512],
                        start=True, stop=False,
                    )
                    for pb in range(PB):
                        nc.tensor.matmul(
                            fin[:ninst, :],
                            pfT[:, pb, :ninst],
                            wo_sb[:, pb, oc * 512:(oc + 1) * 512],
                            start=False, stop=(pb == PB - 1),
                        )
                    nc.vector.tensor_copy(
                        out=osb[:ninst, oc * 512:(oc + 1) * 512],
                        in_=fin[:ninst, :],
                    )
                nc.sync.dma_start(out=out[row0:row0 + ninst, :], in_=osb[:ninst, :])

            at_main = atp.tile([128, DB, H, NMAIN], f32)
            for i in range(NMAIN):
                instance(i, at_main, i)
            group_tail(0, at_main, NMAIN, 0)

            at_aux = atp.tile([128, DB, H, NMAIN], f32)
            for i in range(3):
                instance(NMAIN + i, at_aux, i)
            group_tail(1, at_aux, 3, NMAIN)

    nc.compile()
    return nc


def _get_nc():
    global _NC_CACHE
    if _NC_CACHE is None:
        _NC_CACHE = _build_bass()
    return _NC_CACHE


def _prep_inputs(K, V, query, Wq, bq, Wk, bk, Wv, bv, Wo, bo):
    """Host-side math prep + per-core DMA-friendly packing."""
    K = np.asarray(K, dtype=np.float32)
    V = np.asarray(V, dtype=np.float32)
    query = np.asarray(query, dtype=np.float32)
    Wq = np.asarray(Wq, dtype=np.float32)
    bq = np.asarray(bq, dtype=np.float32)
    Wk = np.asarray(Wk, dtype=np.float32)
    Wv = np.asarray(Wv, dtype=np.float32)
    bv = np.asarray(bv, dtype=np.float32)
    Wo = np.asarray(Wo, dtype=np.float32)
    bo = np.asarray(bo, dtype=np.float32)

    # Qp[g,s,f] = query @ Wq + bq
    qg = query.reshape(G, GS, D)
    Qp = np.einsum("gsd,gdf->gsf", qg, Wq) + bq[:, None, :]
    # wtil[g,s,d,h] = SCALE * sum_e Wk[g,d,h*64+e] * Qp[g,s,h*64+e]
    WkR = Wk.reshape(G, D, H, HD)
    QpR = Qp.reshape(G, GS, H, HD)
    wtil = np.einsum("gdhe,gshe->gsdh", WkR, QpR).astype(np.float32) * np.float32(SCALE)

    # Wo with rows permuted to h-major pooled layout; fold bv into bias
    Wo_p = Wo.reshape(G, HD, H, OD).transpose(0, 2, 1, 3).reshape(G, DD, OD)
    bo_p = bo + np.einsum("gf,gfo->go", bv, Wo_p)

    # packed K^T / V stream: kv_all[b,l] is (128, 4608)
    Kt = np.ascontiguousarray(
        K.reshape(B, L, T, DB, 128).transpose(0, 1, 4, 3, 2)
    ).reshape(B, L, 128, DB * T)
    Vt = np.ascontiguousarray(
        V.reshape(B, L, TB, 128, D).transpose(0, 1, 3, 2, 4)
    ).reshape(B, L, 128, TB * D)

    wv_dev = np.ascontiguousarray(
        Wv.reshape(G, DB, 128, DD).transpose(0, 2, 1, 3)
    )  # (G, 128, DB, DD)
    wo_dev = np.ascontiguousarray(
        Wo_p.reshape(G, PB, 128, OD).transpose(0, 2, 1, 3)
    )  # (G, 128, PB, OD)

    in_maps = []
    inst_rows = []  # per core: list of (b, l) in instance order
    for c in range(NCORES):
        pairs = [(b, 3 * c + s) for b in range(B) for s in range(GS)]
        pairs += [(c, 24 + s) for s in range(GS)]
        bs = np.array([p[0] for p in pairs])
        ls = np.array([p[1] for p in pairs])
        kv_c = np.empty((NI, 128, 2 * 2304), dtype=np.float32)
        kv_c[:, :, :2304] = Kt[bs, ls]
        kv_c[:, :, 2304:] = Vt[bs, ls]

        # wt per instance: main insts use (g=c, s), aux use (g=8, s)
        wt_c = np.empty((128, NI, DB, H), dtype=np.float32)
        for i, (b, l) in enumerate(pairs):
            g, s = divmod(l, GS)
            wt_c[:, i] = wtil[g, s].reshape(DB, 128, H).transpose(1, 0, 2)

        in_maps.append({
            "kv": kv_c,
            "wt": wt_c,
            "wv": np.ascontiguousarray(wv_dev[[c, G - 1]]),
            "wo": np.ascontiguousarray(wo_dev[[c, G - 1]]),
            "bo": np.ascontiguousarray(bo_p[[c, G - 1]]),
        })
        inst_rows.append(pairs)
    return in_maps, inst_rows


def kernel(K, V, query, Wq, bq, Wk, bk, Wv, bv, Wo, bo):
    from concourse.bass_utils import run_bass_kernel_spmd

    nc = _get_nc()
    in_maps, inst_rows = _prep_inputs(K, V, query, Wq, bq, Wk, bk, Wv, bv, Wo, bo)
    res = run_bass_kernel_spmd(nc, in_maps, core_ids=list(range(NCORES)))

    out = np.empty((B, L, OD), dtype=np.float32)
    for c in range(NCORES):
        oc = res.results[c]["out"]
        for i, (b, l) in enumerate(inst_rows[c]):
            out[b, l] = oc[i]
    return out


# revision 3
# speedup vs baseline: 2.1557x; 2.1557x over previous
"""DownsampleExtractor Trainium2 kernel.

Math refactoring (exact up to fp reassociation):
  The reference projects K and V per group (B*L*T rows x 1152 -> 512) and then
  does NQ=1 cross-attention. With a single query per layer this collapses:

  scores[b,l,h,t] = Qp[l,h,:] . Kp[b,l,t,h,:]           (Kp = K @ Wk + bk)
                  = K[b,l,t,:] . (Wk[g] @ Qp_head) + const(l,h)
  The const is invariant over t -> dropped (softmax shift invariance).
  So scores = K[b,l] @ wtil[l]   with wtil[l] = SCALE * Wk[g] @ Qp heads, (1152 x 8).

  pooled[b,l,h,e] = sum_t attn[t] * Vp[t, h*64+e]
                  = (sum_t attn[h,t] V[b,l,t,:]) @ Wv[g][:, h*64+e] + bv  (attn sums to 1)
  So attention is applied to RAW V (A = attn @ V, 8 x 1152), then projected per head.
  This avoids the 130 GFLOP K/V projections entirely (~2.8 GFLOP total).

  The head_dim-major flatten (f = e*8+h) before Wo is handled by row-permuting
  Wo on the host (Wo_p[h*64+e] = Wo[e*8+h]) so the device uses h-major layout.
  bv's contribution is folded into the output bias: bo' = bo + bv @ Wo_p.

Sharding: 72 (b, g) group-instances over 8 cores: core c owns group c for all
8 b (24 layer-instances) plus group 8 for b=c (3 layer-instances). Each core
streams only its own K/V (64 MB) and 2 groups of weights.

All device DMA loads are contiguous-per-partition; the host pre-transposes
K to (d, t) layout (free) and packs K^T and V per layer-instance into one
2.36 MB block so each instance is a single large efficient DMA.
"""

import math

import numpy as np

# hardcoded problem dims
B, L, T, D = 8, 27, 256, 1152
GS = 3
G = L // GS
DD = 512
H, HD = 8, 64
OD = 2048
SCALE = 1.0 / math.sqrt(HD)
NCORES = 8
DB = D // 128   # 9 contraction blocks
TB = T // 128   # 2 token blocks
PB = DD // 128  # 4 blocks of the 512-dim pooled vector
NI = 27         # layer-instances per core (24 main group + 3 aux group)
NMAIN = 24

_NC_CACHE = None


def _build_bass():
    import concourse.bacc as bacc
    import concourse.tile as tile
    import concourse.mybir as mybir
    from concourse.masks import make_identity

    f32 = mybir.dt.float32
    f16 = mybir.dt.float16
    nc = bacc.Bacc(None, target_bir_lowering=False)

    kv = nc.dram_tensor("kv", (NI, 128, 2 * 2304), f16, kind="ExternalInput")
    wt = nc.dram_tensor("wt", (128, NI, DB, H), f16, kind="ExternalInput")
    wv = nc.dram_tensor("wv", (2, 128, DB, DD), f16, kind="ExternalInput")
    wo = nc.dram_tensor("wo", (2, 128, PB, OD), f16, kind="ExternalInput")
    bo = nc.dram_tensor("bo", (2, OD), f16, kind="ExternalInput")
    out = nc.dram_tensor("out", (NI, OD), f32, kind="ExternalOutput")

    with tile.TileContext(nc) as tc:
        with (
            tc.tile_pool(name="const", bufs=1) as const,
            tc.tile_pool(name="kvp", bufs=3) as kvp,
            tc.tile_pool(name="wvp", bufs=1) as wvp,
            tc.tile_pool(name="wop", bufs=1) as wop,
            tc.tile_pool(name="bop", bufs=1) as bop,
            tc.tile_pool(name="atp", bufs=2) as atp,
            tc.tile_pool(name="sm", bufs=3) as sm,
            tc.tile_pool(name="grp", bufs=2) as grp,
            tc.tile_pool(name="ps_sc", bufs=2, space="PSUM") as ps_sc,
            tc.tile_pool(name="ps_tr", bufs=1, space="PSUM") as ps_tr,
            tc.tile_pool(name="ps_at", bufs=2, space="PSUM") as ps_at,
            tc.tile_pool(name="ps_pool", bufs=1, space="PSUM") as ps_pool,
            tc.tile_pool(name="ps_fin", bufs=2, space="PSUM") as ps_fin,
        ):
            ident = const.tile([128, 128], f16)
            make_identity(nc, ident)
            ones = const.tile([1, NMAIN], f16)
            nc.vector.memset(ones, 1.0)

            wt_sb = const.tile([128, NI, DB, H], f16)
            nc.sync.dma_start(out=wt_sb, in_=wt[:, :, :, :])

            def instance(i, at_sb, icol):
                kvt = kvp.tile([128, 2 * 2304], f16)
                nc.sync.dma_start(out=kvt, in_=kv[i, :, :])
                kt = kvt[:, :2304].rearrange("p (db t) -> p db t", db=DB)
                vt = kvt[:, 2304:].rearrange("p (tb d) -> p tb d", tb=TB)

                # scores^T (h x t) = sum_db wtil_block^T.T @ K^T_block
                sc = ps_sc.tile([H, T], f32)
                for db in range(DB):
                    nc.tensor.matmul(
                        sc,
                        wt_sb[:, i, db, :],
                        kt[:, db, :],
                        start=(db == 0),
                        stop=(db == DB - 1),
                    )
                # softmax over t (free dim); logits are O(1) so no max shift
                exps = sm.tile([H, T], f32)
                sums = sm.tile([H, 1], f32)
                nc.scalar.activation(
                    out=exps, in_=sc,
                    func=mybir.ActivationFunctionType.Exp,
                    accum_out=sums,
                )
                rec = sm.tile([H, 1], f32)
                nc.vector.reciprocal(rec, sums)
                attn = sm.tile([H, T], f16)
                nc.vector.tensor_scalar_mul(out=attn, in0=exps, scalar1=rec)

                # attn^T via PE transpose: (8 x 128)->(128 x 8) per t-block
                atr_ps = ps_tr.tile([128, TB, H], f16)
                for tb in range(TB):
                    nc.tensor.transpose(
                        atr_ps[:, tb, :],
                        attn[:, tb * 128:(tb + 1) * 128],
                        ident[:H, :H],
                    )
                attnT = sm.tile([128, TB, H], f16)
                nc.vector.tensor_copy(out=attnT, in_=atr_ps)

                # A^T blocks: (128d x 8h) = V_block(t x d).T @ attn^T(t x h)
                at_ps = ps_at.tile([128, DB, H], f32)
                for db in range(DB):
                    for tb in range(TB):
                        nc.tensor.matmul(
                            at_ps[:, db, :],
                            vt[:, tb, db * 128:(db + 1) * 128],
                            attnT[:, tb, :],
                            start=(tb == 0),
                            stop=(tb == TB - 1),
                        )
                nc.vector.tensor_copy(out=at_sb[:, :, :, icol], in_=at_ps)

            def group_tail(gi, at_sb, ninst, row0):
                wv_sb = wvp.tile([128, DB, DD], f16)
                nc.sync.dma_start(out=wv_sb, in_=wv[gi, :, :, :])
                wo_sb = wop.tile([128, PB, OD], f16)
                nc.sync.dma_start(out=wo_sb, in_=wo[gi, :, :, :])
                bo_sb = bop.tile([1, OD], f16)
                nc.sync.dma_start(out=bo_sb, in_=bo[gi:gi + 1, :])

                # pooled'^T: per head h: (64e x ninst) = Wv_hslice.T @ A^T cols
                pfT = grp.tile([128, PB, NMAIN], f16)
                for h in range(H):
                    pl = ps_pool.tile([64, NMAIN], f32)
                    for db in range(DB):
                        nc.tensor.matmul(
                            pl[:, :ninst],
                            wv_sb[:, db, h * 64:(h + 1) * 64],
                            at_sb[:, db, h, :ninst],
                            start=(db == 0),
                            stop=(db == DB - 1),
                        )
                    nc.vector.tensor_copy(
                        out=pfT[(h % 2) * 64:(h % 2) * 64 + 64, h // 2, :ninst],
                        in_=pl[:, :ninst],
                    )

                # out rows = bo' + sum_pb pfT_block.T @ Wo'_block
                osb = grp.tile([NMAIN, OD], f32)
                for oc in range(OD // 512):
                    fin = ps_fin.tile([NMAIN, 512], f32)
                    nc.tensor.matmul(
                        fin[:ninst, :],
                        ones[:, :ninst],
                        bo_sb[:, oc * 512:(oc + 1) * 512],
                        start=True, stop=False,
                    )
                    for pb in range(PB):
                        nc.tensor.matmul(
                            fin[:ninst, :],
                            pfT[:, pb, :ninst],
                            wo_sb[:, pb, oc * 512:(oc + 1) * 512],
                            start=False, stop=(pb == PB - 1),
                        )
                    nc.vector.tensor_copy(
                        out=osb[:ninst, oc * 512:(oc + 1) * 512],
                        in_=fin[:ninst, :],
                    )
                nc.sync.dma_start(out=out[row0:row0 + ninst, :], in_=osb[:ninst, :])

            at_main = atp.tile([128, DB, H, NMAIN], f16)
            for i in range(NMAIN):
                instance(i, at_main, i)
            group_tail(0, at_main, NMAIN, 0)

            at_aux = atp.tile([128, DB, H, NMAIN], f16)
            for i in range(3):
                instance(NMAIN + i, at_aux, i)
            group_tail(1, at_aux, 3, NMAIN)

    nc.compile()
    return nc


def _get_nc():
    global _NC_CACHE
    if _NC_CACHE is None:
        _NC_CACHE = _build_bass()
    return _NC_CACHE


def _prep_inputs(K, V, query, Wq, bq, Wk, bk, Wv, bv, Wo, bo):
    """Host-side math prep + per-core DMA-friendly packing."""
    K = np.asarray(K, dtype=np.float32)
    V = np.asarray(V, dtype=np.float32)
    query = np.asarray(query, dtype=np.float32)
    Wq = np.asarray(Wq, dtype=np.float32)
    bq = np.asarray(bq, dtype=np.float32)
    Wk = np.asarray(Wk, dtype=np.float32)
    Wv = np.asarray(Wv, dtype=np.float32)
    bv = np.asarray(bv, dtype=np.float32)
    Wo = np.asarray(Wo, dtype=np.float32)
    bo = np.asarray(bo, dtype=np.float32)

    # Qp[g,s,f] = query @ Wq + bq
    qg = query.reshape(G, GS, D)
    Qp = np.einsum("gsd,gdf->gsf", qg, Wq) + bq[:, None, :]
    # wtil[g,s,d,h] = SCALE * sum_e Wk[g,d,h*64+e] * Qp[g,s,h*64+e]
    WkR = Wk.reshape(G, D, H, HD)
    QpR = Qp.reshape(G, GS, H, HD)
    wtil = np.einsum("gdhe,gshe->gsdh", WkR, QpR).astype(np.float32) * np.float32(SCALE)

    # Wo with rows permuted to h-major pooled layout; fold bv into bias
    Wo_p = Wo.reshape(G, HD, H, OD).transpose(0, 2, 1, 3).reshape(G, DD, OD)
    bo_p = bo + np.einsum("gf,gfo->go", bv, Wo_p)

    # packed K^T / V stream: kv_all[b,l] is (128, 4608), fp16 on the wire
    Kt = np.ascontiguousarray(
        K.reshape(B, L, T, DB, 128).transpose(0, 1, 4, 3, 2)
    ).astype(np.float16).reshape(B, L, 128, DB * T)
    Vt = np.ascontiguousarray(
        V.reshape(B, L, TB, 128, D).transpose(0, 1, 3, 2, 4)
    ).astype(np.float16).reshape(B, L, 128, TB * D)

    wv_dev = np.ascontiguousarray(
        Wv.reshape(G, DB, 128, DD).transpose(0, 2, 1, 3)
    ).astype(np.float16)  # (G, 128, DB, DD)
    wo_dev = np.ascontiguousarray(
        Wo_p.reshape(G, PB, 128, OD).transpose(0, 2, 1, 3)
    ).astype(np.float16)  # (G, 128, PB, OD)

    in_maps = []
    inst_rows = []  # per core: list of (b, l) in instance order
    for c in range(NCORES):
        pairs = [(b, 3 * c + s) for b in range(B) for s in range(GS)]
        pairs += [(c, 24 + s) for s in range(GS)]
        bs = np.array([p[0] for p in pairs])
        ls = np.array([p[1] for p in pairs])
        kv_c = np.empty((NI, 128, 2 * 2304), dtype=np.float16)
        kv_c[:, :, :2304] = Kt[bs, ls]
        kv_c[:, :, 2304:] = Vt[bs, ls]

        # wt per instance: main insts use (g=c, s), aux use (g=8, s)
        wt_c = np.empty((128, NI, DB, H), dtype=np.float16)
        for i, (b, l) in enumerate(pairs):
            g, s = divmod(l, GS)
            wt_c[:, i] = wtil[g, s].reshape(DB, 128, H).transpose(1, 0, 2)

        in_maps.append({
            "kv": kv_c,
            "wt": wt_c,
            "wv": np.ascontiguousarray(wv_dev[[c, G - 1]]),
            "wo": np.ascontiguousarray(wo_dev[[c, G - 1]]),
            "bo": np.ascontiguousarray(bo_p[[c, G - 1]]).astype(np.float16),
        })
        inst_rows.append(pairs)
    return in_maps, inst_rows


def kernel(K, V, query, Wq, bq, Wk, bk, Wv, bv, Wo, bo):
    from concourse.bass_utils import run_bass_kernel_spmd

    nc = _get_nc()
    in_maps, inst_rows = _prep_inputs(K, V, query, Wq, bq, Wk, bk, Wv, bv, Wo, bo)
    res = run_bass_kernel_spmd(nc, in_maps, core_ids=list(range(NCORES)))

    out = np.empty((B, L, OD), dtype=np.float32)
    for c in range(NCORES):
        oc = res.results[c]["out"]
        for i, (b, l) in enumerate(inst_rows[c]):
            out[b, l] = oc[i]
    return out


# revision 5
# speedup vs baseline: 2.1814x; 1.0119x over previous
"""DownsampleExtractor Trainium2 kernel.

Math refactoring (exact up to fp reassociation):
  The reference projects K and V per group (B*L*T rows x 1152 -> 512) and then
  does NQ=1 cross-attention. With a single query per layer this collapses:

  scores[b,l,h,t] = Qp[l,h,:] . Kp[b,l,t,h,:]           (Kp = K @ Wk + bk)
                  = K[b,l,t,:] . (Wk[g] @ Qp_head) + const(l,h)
  The const is invariant over t -> dropped (softmax shift invariance).
  So scores = K[b,l] @ wtil[l]   with wtil[l] = SCALE * Wk[g] @ Qp heads, (1152 x 8).

  pooled[b,l,h,e] = sum_t attn[t] * Vp[t, h*64+e]
                  = (sum_t attn[h,t] V[b,l,t,:]) @ Wv[g][:, h*64+e] + bv  (attn sums to 1)
  So attention is applied to RAW V (A = attn @ V, 8 x 1152), then projected per head.
  This avoids the 130 GFLOP K/V projections entirely (~2.8 GFLOP total).

  The head_dim-major flatten (f = e*8+h) before Wo is handled by row-permuting
  Wo on the host (Wo_p[h*64+e] = Wo[e*8+h]) so the device uses h-major layout.
  bv's contribution is folded into the output bias: bo' = bo + bv @ Wo_p.

Sharding: 72 (b, g) group-instances over 8 cores: core c owns group c for all
8 b (24 layer-instances) plus group 8 for b=c (3 layer-instances). Each core
streams only its own K/V (64 MB) and 2 groups of weights.

All device DMA loads are contiguous-per-partition; the host pre-transposes
K to (d, t) layout (free) and packs K^T and V per layer-instance into one
2.36 MB block so each instance is a single large efficient DMA.
"""

import math

import numpy as np

# hardcoded problem dims
B, L, T, D = 8, 27, 256, 1152
GS = 3
G = L // GS
DD = 512
H, HD = 8, 64
OD = 2048
SCALE = 1.0 / math.sqrt(HD)
NCORES = 8
DB = D // 128   # 9 contraction blocks
TB = T // 128   # 2 token blocks
PB = DD // 128  # 4 blocks of the 512-dim pooled vector
NI = 27         # layer-instances per core (24 main group + 3 aux group)
NMAIN = 24

_NC_CACHE = None


def _build_bass():
    import concourse.bacc as bacc
    import concourse.tile as tile
    import concourse.mybir as mybir
    from concourse.masks import make_identity

    f32 = mybir.dt.float32
    f16 = mybir.dt.float16
    nc = bacc.Bacc(None, target_bir_lowering=False)

    kv = nc.dram_tensor("kv", (NI, 128, 2 * 2304), f16, kind="ExternalInput")
    wt = nc.dram_tensor("wt", (128, 2 * GS, DB, H), f16, kind="ExternalInput")
    wv = nc.dram_tensor("wv", (2, 128, DB, DD), f16, kind="ExternalInput")
    wo = nc.dram_tensor("wo", (2, 128, PB, OD), f16, kind="ExternalInput")
    bo = nc.dram_tensor("bo", (2, OD), f16, kind="ExternalInput")
    out = nc.dram_tensor("out", (NI, OD), f32, kind="ExternalOutput")

    with tile.TileContext(nc) as tc:
        with (
            tc.tile_pool(name="const", bufs=1) as const,
            tc.tile_pool(name="kvp", bufs=5) as kvp,
            tc.tile_pool(name="wvp", bufs=1) as wvp,
            tc.tile_pool(name="wop", bufs=1) as wop,
            tc.tile_pool(name="bop", bufs=1) as bop,
            tc.tile_pool(name="atp", bufs=2) as atp,
            tc.tile_pool(name="sm", bufs=4) as sm,
            tc.tile_pool(name="grp", bufs=2) as grp,
            tc.tile_pool(name="ps_sc", bufs=2, space="PSUM") as ps_sc,
            tc.tile_pool(name="ps_tr", bufs=1, space="PSUM") as ps_tr,
            tc.tile_pool(name="ps_at", bufs=2, space="PSUM") as ps_at,
            tc.tile_pool(name="ps_pool", bufs=1, space="PSUM") as ps_pool,
            tc.tile_pool(name="ps_fin", bufs=2, space="PSUM") as ps_fin,
        ):
            ident = const.tile([128, 128], f16)
            make_identity(nc, ident)
            ones = const.tile([1, NMAIN], f16)
            nc.vector.memset(ones, 1.0)

            wt_sb = const.tile([128, 2 * GS, DB, H], f16)
            nc.sync.dma_start(out=wt_sb, in_=wt[:, :, :, :])

            def instance(i, at_sb, icol):
                ws = (3 if i >= NMAIN else 0) + i % GS  # wt slot: (group, s)
                kvt = kvp.tile([128, 2 * 2304], f16)
                nc.sync.dma_start(out=kvt, in_=kv[i, :, :])
                kt = kvt[:, :2304].rearrange("p (db t) -> p db t", db=DB)
                vt = kvt[:, 2304:].rearrange("p (tb d) -> p tb d", tb=TB)

                # scores^T (h x t) = sum_db wtil_block^T.T @ K^T_block
                sc = ps_sc.tile([H, T], f32)
                for db in range(DB):
                    nc.tensor.matmul(
                        sc,
                        wt_sb[:, ws, db, :],
                        kt[:, db, :],
                        start=(db == 0),
                        stop=(db == DB - 1),
                    )
                # softmax over t (free dim); logits are O(1) so no max shift
                exps = sm.tile([H, T], f32)
                sums = sm.tile([H, 1], f32)
                nc.scalar.activation(
                    out=exps, in_=sc,
                    func=mybir.ActivationFunctionType.Exp,
                    accum_out=sums,
                )
                rec = sm.tile([H, 1], f32)
                nc.vector.reciprocal(rec, sums)
                attn = sm.tile([H, T], f16)
                nc.vector.tensor_scalar_mul(out=attn, in0=exps, scalar1=rec)

                # attn^T via PE transpose: (8 x 128)->(128 x 8) per t-block
                atr_ps = ps_tr.tile([128, TB, H], f16)
                for tb in range(TB):
                    nc.tensor.transpose(
                        atr_ps[:, tb, :],
                        attn[:, tb * 128:(tb + 1) * 128],
                        ident[:H, :H],
                    )
                attnT = sm.tile([128, TB, H], f16)
                nc.vector.tensor_copy(out=attnT, in_=atr_ps)

                # A^T blocks: (128d x 8h) = V_block(t x d).T @ attn^T(t x h)
                at_ps = ps_at.tile([128, DB, H], f32)
                for db in range(DB):
                    for tb in range(TB):
                        nc.tensor.matmul(
                            at_ps[:, db, :],
                            vt[:, tb, db * 128:(db + 1) * 128],
                            attnT[:, tb, :],
                            start=(tb == 0),
                            stop=(tb == TB - 1),
                        )
                nc.vector.tensor_copy(out=at_sb[:, :, :, icol], in_=at_ps)

            def group_tail(gi, at_sb, ninst, row0):
                wv_sb = wvp.tile([128, DB, DD], f16)
                nc.sync.dma_start(out=wv_sb, in_=wv[gi, :, :, :])
                wo_sb = wop.tile([128, PB, OD], f16)
                nc.sync.dma_start(out=wo_sb, in_=wo[gi, :, :, :])
                bo_sb = bop.tile([1, OD], f16)
                nc.sync.dma_start(out=bo_sb, in_=bo[gi:gi + 1, :])

                # pooled'^T: per head h: (64e x ninst) = Wv_hslice.T @ A^T cols
                pfT = grp.tile([128, PB, NMAIN], f16)
                for h in range(H):
                    pl = ps_pool.tile([64, NMAIN], f32)
                    for db in range(DB):
                        nc.tensor.matmul(
                            pl[:, :ninst],
                            wv_sb[:, db, h * 64:(h + 1) * 64],
                            at_sb[:, db, h, :ninst],
                            start=(db == 0),
                            stop=(db == DB - 1),
                        )
                    nc.vector.tensor_copy(
                        out=pfT[(h % 2) * 64:(h % 2) * 64 + 64, h // 2, :ninst],
                        in_=pl[:, :ninst],
                    )

                # out rows = bo' + sum_pb pfT_block.T @ Wo'_block
                osb = grp.tile([NMAIN, OD], f32)
                for oc in range(OD // 512):
                    fin = ps_fin.tile([NMAIN, 512], f32)
                    nc.tensor.matmul(
                        fin[:ninst, :],
                        ones[:, :ninst],
                        bo_sb[:, oc * 512:(oc + 1) * 512],
                        start=True, stop=False,
                    )
                    for pb in range(PB):
                        nc.tensor.matmul(
                            fin[:ninst, :],
                            pfT[:, pb, :ninst],
                            wo_sb[:, pb, oc * 512:(oc + 1) * 512],
                            start=False, stop=(pb == PB - 1),
                        )
                    nc.vector.tensor_copy(
                        out=osb[:ninst, oc * 512:(oc + 1) * 512],
                        in_=fin[:ninst, :],
                    )
                nc.sync.dma_start(out=out[row0:row0 + ninst, :], in_=osb[:ninst, :])

            at_main = atp.tile([128, DB, H, NMAIN], f16)
            for i in range(NMAIN):
                instance(i, at_main, i)
            group_tail(0, at_main, NMAIN, 0)

            at_aux = atp.tile([128, DB, H, NMAIN], f16)
            for i in range(3):
                instance(NMAIN + i, at_aux, i)
            group_tail(1, at_aux, 3, NMAIN)

    nc.compile()
    return nc


def _get_nc():
    global _NC_CACHE
    if _NC_CACHE is None:
        _NC_CACHE = _build_bass()
    return _NC_CACHE


def _prep_inputs(K, V, query, Wq, bq, Wk, bk, Wv, bv, Wo, bo):
    """Host-side math prep + per-core DMA-friendly packing."""
    K = np.asarray(K, dtype=np.float32)
    V = np.asarray(V, dtype=np.float32)
    query = np.asarray(query, dtype=np.float32)
    Wq = np.asarray(Wq, dtype=np.float32)
    bq = np.asarray(bq, dtype=np.float32)
    Wk = np.asarray(Wk, dtype=np.float32)
    Wv = np.asarray(Wv, dtype=np.float32)
    bv = np.asarray(bv, dtype=np.float32)
    Wo = np.asarray(Wo, dtype=np.float32)
    bo = np.asarray(bo, dtype=np.float32)

    # Qp[g,s,f] = query @ Wq + bq
    qg = query.reshape(G, GS, D)
    Qp = np.einsum("gsd,gdf->gsf", qg, Wq) + bq[:, None, :]
    # wtil[g,s,d,h] = SCALE * sum_e Wk[g,d,h*64+e] * Qp[g,s,h*64+e]
    WkR = Wk.reshape(G, D, H, HD)
    QpR = Qp.reshape(G, GS, H, HD)
    wtil = np.einsum("gdhe,gshe->gsdh", WkR, QpR).astype(np.float32) * np.float32(SCALE)

    # Wo with rows permuted to h-major pooled layout; fold bv into bias
    Wo_p = Wo.reshape(G, HD, H, OD).transpose(0, 2, 1, 3).reshape(G, DD, OD)
    bo_p = bo + np.einsum("gf,gfo->go", bv, Wo_p)

    # packed K^T / V stream: kv_all[b,l] is (128, 4608), fp16 on the wire
    Kt = np.ascontiguousarray(
        K.reshape(B, L, T, DB, 128).transpose(0, 1, 4, 3, 2)
    ).astype(np.float16).reshape(B, L, 128, DB * T)
    Vt = np.ascontiguousarray(
        V.reshape(B, L, TB, 128, D).transpose(0, 1, 3, 2, 4)
    ).astype(np.float16).reshape(B, L, 128, TB * D)

    wv_dev = np.ascontiguousarray(
        Wv.reshape(G, DB, 128, DD).transpose(0, 2, 1, 3)
    ).astype(np.float16)  # (G, 128, DB, DD)
    wo_dev = np.ascontiguousarray(
        Wo_p.reshape(G, PB, 128, OD).transpose(0, 2, 1, 3)
    ).astype(np.float16)  # (G, 128, PB, OD)

    in_maps = []
    inst_rows = []  # per core: list of (b, l) in instance order
    for c in range(NCORES):
        pairs = [(b, 3 * c + s) for b in range(B) for s in range(GS)]
        pairs += [(c, 24 + s) for s in range(GS)]
        bs = np.array([p[0] for p in pairs])
        ls = np.array([p[1] for p in pairs])
        kv_c = np.empty((NI, 128, 2 * 2304), dtype=np.float16)
        kv_c[:, :, :2304] = Kt[bs, ls]
        kv_c[:, :, 2304:] = Vt[bs, ls]

        # wt slots: 3 for the main group (g=c), 3 for the aux group (g=8)
        wt_c = np.empty((128, 2 * GS, DB, H), dtype=np.float16)
        for j, g in enumerate((c, G - 1)):
            for s in range(GS):
                wt_c[:, j * GS + s] = wtil[g, s].reshape(DB, 128, H).transpose(1, 0, 2)

        in_maps.append({
            "kv": kv_c,
            "wt": wt_c,
            "wv": np.ascontiguousarray(wv_dev[[c, G - 1]]),
            "wo": np.ascontiguousarray(wo_dev[[c, G - 1]]),
            "bo": np.ascontiguousarray(bo_p[[c, G - 1]]).astype(np.float16),
        })
        inst_rows.append(pairs)
    return in_maps, inst_rows


def kernel(K, V, query, Wq, bq, Wk, bk, Wv, bv, Wo, bo):
    from concourse.bass_utils import run_bass_kernel_spmd

    nc = _get_nc()
    in_maps, inst_rows = _prep_inputs(K, V, query, Wq, bq, Wk, bk, Wv, bv, Wo, bo)
    res = run_bass_kernel_spmd(nc, in_maps, core_ids=list(range(NCORES)))

    out = np.empty((B, L, OD), dtype=np.float32)
    for c in range(NCORES):
        oc = res.results[c]["out"]
        for i, (b, l) in enumerate(inst_rows[c]):
            out[b, l] = oc[i]
    return out


# revision 11
# speedup vs baseline: 2.2212x; 1.0182x over previous
"""DownsampleExtractor Trainium2 kernel.

Math refactoring (exact up to fp reassociation):
  The reference projects K and V per group (B*L*T rows x 1152 -> 512) and then
  does NQ=1 cross-attention. With a single query per layer this collapses:

  scores[b,l,h,t] = Qp[l,h,:] . Kp[b,l,t,h,:]           (Kp = K @ Wk + bk)
                  = K[b,l,t,:] . (Wk[g] @ Qp_head) + const(l,h)
  The const is invariant over t -> dropped (softmax shift invariance).
  So scores = K[b,l] @ wtil[l]   with wtil[l] = SCALE * Wk[g] @ Qp heads, (1152 x 8).

  pooled[b,l,h,e] = sum_t attn[t] * Vp[t, h*64+e]
                  = (sum_t attn[h,t] V[b,l,t,:]) @ Wv[g][:, h*64+e] + bv  (attn sums to 1)
  So attention is applied to RAW V (A = attn @ V, 8 x 1152), then projected per head.
  This avoids the 130 GFLOP K/V projections entirely (~2.8 GFLOP total).

  The head_dim-major flatten (f = e*8+h) before Wo is handled by row-permuting
  Wo on the host (Wo_p[h*64+e] = Wo[e*8+h]) so the device uses h-major layout.
  bv's contribution is folded into the output bias: bo' = bo + bv @ Wo_p.

Sharding: 72 (b, g) group-instances over 8 cores: core c owns group c for all
8 b (24 layer-instances) plus group 8 for b=c (3 layer-instances). Each core
streams only its own K/V (64 MB) and 2 groups of weights.

All device DMA loads are contiguous-per-partition; the host pre-transposes
K to (d, t) layout (free) and packs K^T and V per layer-instance into one
2.36 MB block so each instance is a single large efficient DMA.
"""

import math

import numpy as np

# hardcoded problem dims
B, L, T, D = 8, 27, 256, 1152
GS = 3
G = L // GS
DD = 512
H, HD = 8, 64
OD = 2048
SCALE = 1.0 / math.sqrt(HD)
NCORES = 8
DB = D // 128   # 9 contraction blocks
TB = T // 128   # 2 token blocks
PB = DD // 128  # 4 blocks of the 512-dim pooled vector
NI = 27         # layer-instances per core (24 main group + 3 aux group)
NMAIN = 24

_NC_CACHE = None


def _build_bass():
    import concourse.bacc as bacc
    import concourse.tile as tile
    import concourse.mybir as mybir
    from concourse.masks import make_identity

    f32 = mybir.dt.float32
    f16 = mybir.dt.float16
    nc = bacc.Bacc(None, target_bir_lowering=False)

    kv = nc.dram_tensor("kv", (NI, 128, 2 * 2304), f16, kind="ExternalInput")
    wt = nc.dram_tensor("wt", (128, 2 * GS, DB, H), f16, kind="ExternalInput")
    wv = nc.dram_tensor("wv", (2, 128, DB, DD), f16, kind="ExternalInput")
    wo = nc.dram_tensor("wo", (2, OD // 512, 128, PB, 512), f16, kind="ExternalInput")
    bo = nc.dram_tensor("bo", (2, OD), f16, kind="ExternalInput")
    out = nc.dram_tensor("out", (NI, OD), f32, kind="ExternalOutput")

    with tile.TileContext(nc) as tc:
        with (
            tc.tile_pool(name="const", bufs=1) as const,
            tc.tile_pool(name="kvp", bufs=10) as kvp,
            tc.tile_pool(name="wvp", bufs=2) as wvp,
            tc.tile_pool(name="wop", bufs=8) as wop,
            tc.tile_pool(name="bop", bufs=2) as bop,
            tc.tile_pool(name="atp", bufs=2) as atp,
            tc.tile_pool(name="sm", bufs=4) as sm,
            tc.tile_pool(name="grp", bufs=2) as grp,
            tc.tile_pool(name="ps_sc", bufs=2, space="PSUM") as ps_sc,
            tc.tile_pool(name="ps_tr", bufs=1, space="PSUM") as ps_tr,
            tc.tile_pool(name="ps_at", bufs=2, space="PSUM") as ps_at,
            tc.tile_pool(name="ps_pool", bufs=1, space="PSUM") as ps_pool,
            tc.tile_pool(name="ps_fin", bufs=2, space="PSUM") as ps_fin,
        ):
            ident = const.tile([128, 128], f16)
            make_identity(nc, ident)
            ones = const.tile([1, NMAIN], f16)
            nc.vector.memset(ones, 1.0)

            wt_sb = const.tile([128, 2 * GS, DB, H], f16)
            nc.sync.dma_start(out=wt_sb, in_=wt[:, :, :, :])

            def load_k(i):
                ktile = kvp.tile([128, 2304], f16, tag="kvt")
                nc.sync.dma_start(out=ktile, in_=kv[i, :, :2304])
                return ktile

            def load_v(i):
                vtile = kvp.tile([128, 2304], f16, tag="kvt")
                nc.sync.dma_start(out=vtile, in_=kv[i, :, 2304:])
                return vtile

            def instance(i, at_sb, icol, ktile=None, vtile=None):
                ws = (3 if i >= NMAIN else 0) + i % GS  # wt slot: (group, s)
                if ktile is None:
                    ktile = load_k(i)
                if vtile is None:
                    vtile = load_v(i)
                kt = ktile.rearrange("p (db t) -> p db t", db=DB)
                vt = vtile.rearrange("p (tb d) -> p tb d", tb=TB)

                # scores^T (h x t) = sum_db wtil_block^T.T @ K^T_block
                sc = ps_sc.tile([H, T], f32)
                for db in range(DB):
                    nc.tensor.matmul(
                        sc,
                        wt_sb[:, ws, db, :],
                        kt[:, db, :],
                        start=(db == 0),
                        stop=(db == DB - 1),
                    )
                # softmax over t (free dim); logits are O(1) so no max shift
                exps = sm.tile([H, T], f32)
                sums = sm.tile([H, 1], f32)
                nc.scalar.activation(
                    out=exps, in_=sc,
                    func=mybir.ActivationFunctionType.Exp,
                    accum_out=sums,
                )
                rec = sm.tile([H, 1], f32)
                nc.vector.reciprocal(rec, sums)
                attn = sm.tile([H, T], f16)
                nc.vector.tensor_scalar_mul(out=attn, in0=exps, scalar1=rec)

                # attn^T via PE transpose: (8 x 128)->(128 x 8) per t-block
                atr_ps = ps_tr.tile([128, TB, H], f16)
                for tb in range(TB):
                    nc.tensor.transpose(
                        atr_ps[:, tb, :],
                        attn[:, tb * 128:(tb + 1) * 128],
                        ident[:H, :H],
                    )
                attnT = sm.tile([128, TB, H], f16)
                nc.vector.tensor_copy(out=attnT, in_=atr_ps)

                # A^T blocks: (128d x 8h) = V_block(t x d).T @ attn^T(t x h)
                at_ps = ps_at.tile([128, DB, H], f32)
                for db in range(DB):
                    for tb in range(TB):
                        nc.tensor.matmul(
                            at_ps[:, db, :],
                            vt[:, tb, db * 128:(db + 1) * 128],
                            attnT[:, tb, :],
                            start=(tb == 0),
                            stop=(tb == TB - 1),
                        )
                nc.vector.tensor_copy(out=at_sb[:, :, :, icol], in_=at_ps)

            def load_group_weights(gi):
                # wv first (pooled consumes it first), wo last: the weight
                # stream doubles as DMA cover for the attention-apply chain
                wv_sb = wvp.tile([128, DB, DD], f16)
                nc.sync.dma_start(out=wv_sb, in_=wv[gi, :, :, :])
                bo_sb = bop.tile([1, OD], f16)
                nc.sync.dma_start(out=bo_sb, in_=bo[gi:gi + 1, :])
                wo_q = []
                for oc in range(OD // 512):
                    wq = wop.tile([128, PB, 512], f16, tag="woq")
                    nc.sync.dma_start(out=wq, in_=wo[gi, oc, :, :, :])
                    wo_q.append(wq)
                return wv_sb, wo_q, bo_sb

            def group_tail(gtiles, at_sb, ninst, row0):
                wv_sb, wo_q, bo_sb = gtiles

                # pooled'^T, full-product form: per f'-block pb (= heads 2pb,2pb+1)
                # F[p, h', inst] = sum_d Wv[d, pb*128+p] * A^T[d, inst, 2pb+h'];
                # the needed rows are the h(p) "diagonal": h' = p//64.
                pfT = grp.tile([128, PB, NMAIN], f16)
                for pb in range(PB):
                    pl = ps_pool.tile([128, 2, NMAIN], f32)
                    for db in range(DB):
                        nc.tensor.matmul(
                            pl[:, :, :ninst],
                            wv_sb[:, db, pb * 128:(pb + 1) * 128],
                            at_sb[:, db, 2 * pb:2 * pb + 2, :ninst],
                            start=(db == 0),
                            stop=(db == DB - 1),
                        )
                    nc.vector.tensor_copy(
                        out=pfT[0:64, pb, :ninst], in_=pl[0:64, 0, :ninst])
                    nc.vector.tensor_copy(
                        out=pfT[64:128, pb, :ninst], in_=pl[64:128, 1, :ninst])

                # out rows = bo' + sum_pb pfT_block.T @ Wo'_block
                osb = grp.tile([NMAIN, OD], f32)
                for oc in range(OD // 512):
                    fin = ps_fin.tile([NMAIN, 512], f32)
                    nc.tensor.matmul(
                        fin[:ninst, :],
                        ones[:, :ninst],
                        bo_sb[:, oc * 512:(oc + 1) * 512],
                        start=True, stop=False,
                    )
                    for pb in range(PB):
                        nc.tensor.matmul(
                            fin[:ninst, :],
                            pfT[:, pb, :ninst],
                            wo_q[oc][:, pb, :],
                            start=False, stop=(pb == PB - 1),
                        )
                    nc.vector.tensor_copy(
                        out=osb[:ninst, oc * 512:(oc + 1) * 512],
                        in_=fin[:ninst, :],
                    )
                nc.sync.dma_start(out=out[row0:row0 + ninst, :], in_=osb[:ninst, :])

            at_main = atp.tile([128, DB, H, NMAIN], f16)
            for i in range(NMAIN):
                instance(i, at_main, i)
            gw_main = load_group_weights(0)
            group_tail(gw_main, at_main, NMAIN, 0)

            # aux group: hoist instance 26's K and load aux weights before its
            # V so that after the final byte only the attention-apply matmuls
            # and the (small) aux tail remain
            at_aux = atp.tile([128, DB, H, NMAIN], f16)
            k26 = load_k(NMAIN + 2)
            instance(NMAIN + 0, at_aux, 0)
            instance(NMAIN + 1, at_aux, 1)
            v26 = load_v(NMAIN + 2)
            instance(NMAIN + 2, at_aux, 2, ktile=k26, vtile=v26)
            gw_aux = load_group_weights(1)
            group_tail(gw_aux, at_aux, 3, NMAIN)

    nc.compile()
    return nc


def _get_nc():
    global _NC_CACHE
    if _NC_CACHE is None:
        _NC_CACHE = _build_bass()
    return _NC_CACHE


def _prep_inputs(K, V, query, Wq, bq, Wk, bk, Wv, bv, Wo, bo):
    """Host-side math prep + per-core DMA-friendly packing."""
    K = np.asarray(K, dtype=np.float32)
    V = np.asarray(V, dtype=np.float32)
    query = np.asarray(query, dtype=np.float32)
    Wq = np.asarray(Wq, dtype=np.float32)
    bq = np.asarray(bq, dtype=np.float32)
    Wk = np.asarray(Wk, dtype=np.float32)
    Wv = np.asarray(Wv, dtype=np.float32)
    bv = np.asarray(bv, dtype=np.float32)
    Wo = np.asarray(Wo, dtype=np.float32)
    bo = np.asarray(bo, dtype=np.float32)

    # Qp[g,s,f] = query @ Wq + bq
    qg = query.reshape(G, GS, D)
    Qp = np.einsum("gsd,gdf->gsf", qg, Wq) + bq[:, None, :]
    # wtil[g,s,d,h] = SCALE * sum_e Wk[g,d,h*64+e] * Qp[g,s,h*64+e]
    WkR = Wk.reshape(G, D, H, HD)
    QpR = Qp.reshape(G, GS, H, HD)
    wtil = np.einsum("gdhe,gshe->gsdh", WkR, QpR).astype(np.float32) * np.float32(SCALE)

    # Wo with rows permuted to h-major pooled layout; fold bv into bias
    Wo_p = Wo.reshape(G, HD, H, OD).transpose(0, 2, 1, 3).reshape(G, DD, OD)
    bo_p = bo + np.einsum("gf,gfo->go", bv, Wo_p)

    # packed K^T / V stream: kv_all[b,l] is (128, 4608), fp16 on the wire
    Kt = np.ascontiguousarray(
        K.reshape(B, L, T, DB, 128).transpose(0, 1, 4, 3, 2)
    ).astype(np.float16).reshape(B, L, 128, DB * T)
    Vt = np.ascontiguousarray(
        V.reshape(B, L, TB, 128, D).transpose(0, 1, 3, 2, 4)
    ).astype(np.float16).reshape(B, L, 128, TB * D)

    wv_dev = np.ascontiguousarray(
        Wv.reshape(G, DB, 128, DD).transpose(0, 2, 1, 3)
    ).astype(np.float16)  # (G, 128, DB, DD)
    wo_dev = np.ascontiguousarray(
        Wo_p.reshape(G, PB, 128, OD // 512, 512).transpose(0, 3, 2, 1, 4)
    ).astype(np.float16)  # (G, OC, 128, PB, 512)

    in_maps = []
    inst_rows = []  # per core: list of (b, l) in instance order
    for c in range(NCORES):
        pairs = [(b, 3 * c + s) for b in range(B) for s in range(GS)]
        pairs += [(c, 24 + s) for s in range(GS)]
        bs = np.array([p[0] for p in pairs])
        ls = np.array([p[1] for p in pairs])
        kv_c = np.empty((NI, 128, 2 * 2304), dtype=np.float16)
        kv_c[:, :, :2304] = Kt[bs, ls]
        kv_c[:, :, 2304:] = Vt[bs, ls]

        # wt slots: 3 for the main group (g=c), 3 for the aux group (g=8)
        wt_c = np.empty((128, 2 * GS, DB, H), dtype=np.float16)
        for j, g in enumerate((c, G - 1)):
            for s in range(GS):
                wt_c[:, j * GS + s] = wtil[g, s].reshape(DB, 128, H).transpose(1, 0, 2)

        in_maps.append({
            "kv": kv_c,
            "wt": wt_c,
            "wv": np.ascontiguousarray(wv_dev[[c, G - 1]]),
            "wo": np.ascontiguousarray(wo_dev[[c, G - 1]]),
            "bo": np.ascontiguousarray(bo_p[[c, G - 1]]).astype(np.float16),
        })
        inst_rows.append(pairs)
    return in_maps, inst_rows


def kernel(K, V, query, Wq, bq, Wk, bk, Wv, bv, Wo, bo):
    from concourse.bass_utils import run_bass_kernel_spmd

    nc = _get_nc()
    in_maps, inst_rows = _prep_inputs(K, V, query, Wq, bq, Wk, bk, Wv, bv, Wo, bo)
    res = run_bass_kernel_spmd(nc, in_maps, core_ids=list(range(NCORES)))

    out = np.empty((B, L, OD), dtype=np.float32)
    for c in range(NCORES):
        oc = res.results[c]["out"]
        for i, (b, l) in enumerate(inst_rows[c]):
            out[b, l] = oc[i]
    return out


# revision 15
# speedup vs baseline: 2.2277x; 1.0029x over previous
"""DownsampleExtractor Trainium2 kernel.

Math refactoring (exact up to fp reassociation):
  The reference projects K and V per group (B*L*T rows x 1152 -> 512) and then
  does NQ=1 cross-attention. With a single query per layer this collapses:

  scores[b,l,h,t] = Qp[l,h,:] . Kp[b,l,t,h,:]           (Kp = K @ Wk + bk)
                  = K[b,l,t,:] . (Wk[g] @ Qp_head) + const(l,h)
  The const is invariant over t -> dropped (softmax shift invariance).
  So scores = K[b,l] @ wtil[l]   with wtil[l] = SCALE * Wk[g] @ Qp heads, (1152 x 8).

  pooled[b,l,h,e] = sum_t attn[t] * Vp[t, h*64+e]
                  = (sum_t attn[h,t] V[b,l,t,:]) @ Wv[g][:, h*64+e] + bv  (attn sums to 1)
  So attention is applied to RAW V (A = attn @ V, 8 x 1152), then projected per head.
  This avoids the 130 GFLOP K/V projections entirely (~2.8 GFLOP total).

  The head_dim-major flatten (f = e*8+h) before Wo is handled by row-permuting
  Wo on the host (Wo_p[h*64+e] = Wo[e*8+h]) so the device uses h-major layout.
  bv's contribution is folded into the output bias: bo' = bo + bv @ Wo_p.

Sharding: 72 (b, g) group-instances over 8 cores: core c owns group c for all
8 b (24 layer-instances) plus group 8 for b=c (3 layer-instances). Each core
streams only its own K/V and 2 groups of weights (~19.5 MB in fp16).

Precision: everything DMA'd is fp16 (all values are O(1), so fp16's 11-bit
mantissa beats bf16 by ~8x at the same size; fp32 matmul would also run at
1/4 PE rate). Accumulation, softmax and the output stay fp32. Measured
max-rel error vs the fp32 reference: 5.2e-4 (fp32 variant: 3.8e-6).

All device DMA loads are contiguous-per-partition; the host pre-transposes
K to (d, t) layout and packs K^T and V per layer-instance. The kernel is
DMA-roofline bound (~108us of ~116us cost-model time per core); the stream
is ordered so every compute tail except the last ~5us has DMA cover.
"""

import math

import numpy as np

# hardcoded problem dims
B, L, T, D = 8, 27, 256, 1152
GS = 3
G = L // GS
DD = 512
H, HD = 8, 64
OD = 2048
SCALE = 1.0 / math.sqrt(HD)
NCORES = 8
DB = D // 128   # 9 contraction blocks
TB = T // 128   # 2 token blocks
PB = DD // 128  # 4 blocks of the 512-dim pooled vector
NI = 27         # layer-instances per core (24 main group + 3 aux group)
NMAIN = 24

_NC_CACHE = None


def _build_bass():
    import concourse.bacc as bacc
    import concourse.tile as tile
    import concourse.mybir as mybir
    from concourse.masks import make_identity

    f32 = mybir.dt.float32
    f16 = mybir.dt.float16
    nc = bacc.Bacc(None, target_bir_lowering=False)

    kv = nc.dram_tensor("kv", (NI, 128, 2 * 2304), f16, kind="ExternalInput")
    wt = nc.dram_tensor("wt", (128, 2 * GS, DB, H), f16, kind="ExternalInput")
    wv = nc.dram_tensor("wv", (2, 128, DB, DD), f16, kind="ExternalInput")
    wo = nc.dram_tensor("wo", (2, OD // 512, 128, PB, 512), f16, kind="ExternalInput")
    bo = nc.dram_tensor("bo", (2, OD), f16, kind="ExternalInput")
    out = nc.dram_tensor("out", (NI, OD), f32, kind="ExternalOutput")

    with tile.TileContext(nc) as tc:
        with (
            tc.tile_pool(name="const", bufs=1) as const,
            tc.tile_pool(name="kvp", bufs=10) as kvp,
            tc.tile_pool(name="wvp", bufs=2) as wvp,
            tc.tile_pool(name="wop", bufs=8) as wop,
            tc.tile_pool(name="bop", bufs=2) as bop,
            tc.tile_pool(name="atp", bufs=2) as atp,
            tc.tile_pool(name="sm", bufs=4) as sm,
            tc.tile_pool(name="grp", bufs=2) as grp,
            tc.tile_pool(name="ps_sc", bufs=2, space="PSUM") as ps_sc,
            tc.tile_pool(name="ps_tr", bufs=1, space="PSUM") as ps_tr,
            tc.tile_pool(name="ps_at", bufs=2, space="PSUM") as ps_at,
            tc.tile_pool(name="ps_pool", bufs=1, space="PSUM") as ps_pool,
            tc.tile_pool(name="ps_fin", bufs=2, space="PSUM") as ps_fin,
        ):
            ident = const.tile([128, 128], f16)
            make_identity(nc, ident)
            ones = const.tile([1, NMAIN], f16)
            nc.vector.memset(ones, 1.0)


            def load_k(i):
                ktile = kvp.tile([128, 2304], f16, tag="kvt")
                nc.sync.dma_start(out=ktile, in_=kv[i, :, :2304])
                return ktile

            def load_v(i):
                vtile = kvp.tile([128, 2304], f16, tag="kvt")
                nc.sync.dma_start(out=vtile, in_=kv[i, :, 2304:])
                return vtile

            def instance(i, at_sb, icol, ktile=None, vtile=None):
                ws = (3 if i >= NMAIN else 0) + i % GS  # wt slot: (group, s)
                if ktile is None:
                    ktile = load_k(i)
                if vtile is None:
                    vtile = load_v(i)
                kt = ktile.rearrange("p (db t) -> p db t", db=DB)
                vt = vtile.rearrange("p (tb d) -> p tb d", tb=TB)

                # scores^T (h x t) = sum_db wtil_block^T.T @ K^T_block
                sc = ps_sc.tile([H, T], f32)
                for db in range(DB):
                    nc.tensor.matmul(
                        sc,
                        wt_sb[:, ws, db, :],
                        kt[:, db, :],
                        start=(db == 0),
                        stop=(db == DB - 1),
                    )
                # softmax over t (free dim); logits are O(1) so no max shift
                exps = sm.tile([H, T], f32)
                sums = sm.tile([H, 1], f32)
                nc.scalar.activation(
                    out=exps, in_=sc,
                    func=mybir.ActivationFunctionType.Exp,
                    accum_out=sums,
                )
                rec = sm.tile([H, 1], f32)
                nc.vector.reciprocal(rec, sums)
                attn = sm.tile([H, T], f16)
                nc.vector.tensor_scalar_mul(out=attn, in0=exps, scalar1=rec)

                # attn^T via PE transpose: (8 x 128)->(128 x 8) per t-block
                atr_ps = ps_tr.tile([128, TB, H], f16)
                for tb in range(TB):
                    nc.tensor.transpose(
                        atr_ps[:, tb, :],
                        attn[:, tb * 128:(tb + 1) * 128],
                        ident[:H, :H],
                    )
                attnT = sm.tile([128, TB, H], f16)
                nc.vector.tensor_copy(out=attnT, in_=atr_ps)

                # A^T blocks: (128d x 8h) = V_block(t x d).T @ attn^T(t x h)
                at_ps = ps_at.tile([128, DB, H], f32)
                for db in range(DB):
                    for tb in range(TB):
                        nc.tensor.matmul(
                            at_ps[:, db, :],
                            vt[:, tb, db * 128:(db + 1) * 128],
                            attnT[:, tb, :],
                            start=(tb == 0),
                            stop=(tb == TB - 1),
                        )
                nc.vector.tensor_copy(out=at_sb[:, :, :, icol], in_=at_ps)

            def load_group_weights(gi):
                # wv first (pooled consumes it first), wo last: the weight
                # stream doubles as DMA cover for the attention-apply chain
                wv_sb = wvp.tile([128, DB, DD], f16)
                nc.sync.dma_start(out=wv_sb, in_=wv[gi, :, :, :])
                bo_sb = bop.tile([1, OD], f16)
                nc.sync.dma_start(out=bo_sb, in_=bo[gi:gi + 1, :])
                wo_q = []
                for oc in range(OD // 512):
                    wq = wop.tile([128, PB, 512], f16, tag="woq")
                    nc.sync.dma_start(out=wq, in_=wo[gi, oc, :, :, :])
                    wo_q.append(wq)
                return wv_sb, wo_q, bo_sb

            def group_tail(gtiles, at_sb, ninst, row0):
                wv_sb, wo_q, bo_sb = gtiles

                # pooled'^T, full-product form: per f'-block pb (= heads 2pb,2pb+1)
                # F[p, h', inst] = sum_d Wv[d, pb*128+p] * A^T[d, inst, 2pb+h'];
                # the needed rows are the h(p) "diagonal": h' = p//64.
                pfT = grp.tile([128, PB, NMAIN], f16)
                for pb in range(PB):
                    pl = ps_pool.tile([128, 2, NMAIN], f32)
                    for db in range(DB):
                        nc.tensor.matmul(
                            pl[:, :, :ninst],
                            wv_sb[:, db, pb * 128:(pb + 1) * 128],
                            at_sb[:, db, 2 * pb:2 * pb + 2, :ninst],
                            start=(db == 0),
                            stop=(db == DB - 1),
                        )
                    nc.vector.tensor_copy(
                        out=pfT[0:64, pb, :ninst], in_=pl[0:64, 0, :ninst])
                    nc.vector.tensor_copy(
                        out=pfT[64:128, pb, :ninst], in_=pl[64:128, 1, :ninst])

                # out rows = bo' + sum_pb pfT_block.T @ Wo'_block
                osb = grp.tile([NMAIN, OD], f32)
                for oc in range(OD // 512):
                    fin = ps_fin.tile([NMAIN, 512], f32)
                    nc.tensor.matmul(
                        fin[:ninst, :],
                        ones[:, :ninst],
                        bo_sb[:, oc * 512:(oc + 1) * 512],
                        start=True, stop=False,
                    )
                    for pb in range(PB):
                        nc.tensor.matmul(
                            fin[:ninst, :],
                            pfT[:, pb, :ninst],
                            wo_q[oc][:, pb, :],
                            start=False, stop=(pb == PB - 1),
                        )
                    nc.vector.tensor_copy(
                        out=osb[:ninst, oc * 512:(oc + 1) * 512],
                        in_=fin[:ninst, :],
                    )
                nc.sync.dma_start(out=out[row0:row0 + ninst, :], in_=osb[:ninst, :])

            at_main = atp.tile([128, DB, H, NMAIN], f16)
            k0 = load_k(0)
            wt_sb = const.tile([128, 2 * GS, DB, H], f16)
            nc.sync.dma_start(out=wt_sb, in_=wt[:, :, :, :])
            instance(0, at_main, 0, ktile=k0)
            for i in range(1, NMAIN):
                instance(i, at_main, i)
            gw_main = load_group_weights(0)
            group_tail(gw_main, at_main, NMAIN, 0)

            # aux group last (small tail): instance 26's K is hoisted so its
            # scores can run early, V26 is the last K/V transfer, and the aux
            # weight stream after it covers the attention-apply + pooled
            # chain; only the final projection trails the last byte
            at_aux = atp.tile([128, DB, H, NMAIN], f16)
            k26 = load_k(NMAIN + 2)
            instance(NMAIN + 0, at_aux, 0)
            instance(NMAIN + 1, at_aux, 1)
            v26 = load_v(NMAIN + 2)
            instance(NMAIN + 2, at_aux, 2, ktile=k26, vtile=v26)
            gw_aux = load_group_weights(1)
            group_tail(gw_aux, at_aux, 3, NMAIN)

    nc.compile()
    return nc


def _get_nc():
    global _NC_CACHE
    if _NC_CACHE is None:
        _NC_CACHE = _build_bass()
    return _NC_CACHE


def _prep_inputs(K, V, query, Wq, bq, Wk, bk, Wv, bv, Wo, bo):
    """Host-side math prep + per-core DMA-friendly packing."""
    K = np.asarray(K, dtype=np.float32)
    V = np.asarray(V, dtype=np.float32)
    query = np.asarray(query, dtype=np.float32)
    Wq = np.asarray(Wq, dtype=np.float32)
    bq = np.asarray(bq, dtype=np.float32)
    Wk = np.asarray(Wk, dtype=np.float32)
    Wv = np.asarray(Wv, dtype=np.float32)
    bv = np.asarray(bv, dtype=np.float32)
    Wo = np.asarray(Wo, dtype=np.float32)
    bo = np.asarray(bo, dtype=np.float32)

    # Qp[g,s,f] = query @ Wq + bq
    qg = query.reshape(G, GS, D)
    Qp = np.einsum("gsd,gdf->gsf", qg, Wq) + bq[:, None, :]
    # wtil[g,s,d,h] = SCALE * sum_e Wk[g,d,h*64+e] * Qp[g,s,h*64+e]
    WkR = Wk.reshape(G, D, H, HD)
    QpR = Qp.reshape(G, GS, H, HD)
    wtil = np.einsum("gdhe,gshe->gsdh", WkR, QpR).astype(np.float32) * np.float32(SCALE)

    # Wo with rows permuted to h-major pooled layout; fold bv into bias
    Wo_p = Wo.reshape(G, HD, H, OD).transpose(0, 2, 1, 3).reshape(G, DD, OD)
    bo_p = bo + np.einsum("gf,gfo->go", bv, Wo_p)

    # packed K^T / V stream: kv_all[b,l] is (128, 4608), fp16 on the wire
    Kt = np.ascontiguousarray(
        K.reshape(B, L, T, DB, 128).transpose(0, 1, 4, 3, 2)
    ).astype(np.float16).reshape(B, L, 128, DB * T)
    Vt = np.ascontiguousarray(
        V.reshape(B, L, TB, 128, D).transpose(0, 1, 3, 2, 4)
    ).astype(np.float16).reshape(B, L, 128, TB * D)

    wv_dev = np.ascontiguousarray(
        Wv.reshape(G, DB, 128, DD).transpose(0, 2, 1, 3)
    ).astype(np.float16)  # (G, 128, DB, DD)
    wo_dev = np.ascontiguousarray(
        Wo_p.reshape(G, PB, 128, OD // 512, 512).transpose(0, 3, 2, 1, 4)
    ).astype(np.float16)  # (G, OC, 128, PB, 512)

    in_maps = []
    inst_rows = []  # per core: list of (b, l) in instance order
    for c in range(NCORES):
        pairs = [(b, 3 * c + s) for b in range(B) for s in range(GS)]
        pairs += [(c, 24 + s) for s in range(GS)]
        bs = np.array([p[0] for p in pairs])
        ls = np.array([p[1] for p in pairs])
        kv_c = np.empty((NI, 128, 2 * 2304), dtype=np.float16)
        kv_c[:, :, :2304] = Kt[bs, ls]
        kv_c[:, :, 2304:] = Vt[bs, ls]

        # wt slots: 3 for the main group (g=c), 3 for the aux group (g=8)
        wt_c = np.empty((128, 2 * GS, DB, H), dtype=np.float16)
        for j, g in enumerate((c, G - 1)):
            for s in range(GS):
                wt_c[:, j * GS + s] = wtil[g, s].reshape(DB, 128, H).transpose(1, 0, 2)

        in_maps.append({
            "kv": kv_c,
            "wt": wt_c,
            "wv": np.ascontiguousarray(wv_dev[[c, G - 1]]),
            "wo": np.ascontiguousarray(wo_dev[[c, G - 1]]),
            "bo": np.ascontiguousarray(bo_p[[c, G - 1]]).astype(np.float16),
        })
        inst_rows.append(pairs)
    return in_maps, inst_rows


def kernel(K, V, query, Wq, bq, Wk, bk, Wv, bv, Wo, bo):
    from concourse.bass_utils import run_bass_kernel_spmd

    nc = _get_nc()
    in_maps, inst_rows = _prep_inputs(K, V, query, Wq, bq, Wk, bk, Wv, bv, Wo, bo)
    res = run_bass_kernel_spmd(nc, in_maps, core_ids=list(range(NCORES)))

    out = np.empty((B, L, OD), dtype=np.float32)
    for c in range(NCORES):
        oc = res.results[c]["out"]
        for i, (b, l) in enumerate(inst_rows[c]):
            out[b, l] = oc[i]
    return out


# revision 18
# speedup vs baseline: 2.2387x; 1.0050x over previous
"""DownsampleExtractor Trainium2 kernel.

Math refactoring (exact up to fp reassociation):
  The reference projects K and V per group (B*L*T rows x 1152 -> 512) and then
  does NQ=1 cross-attention. With a single query per layer this collapses:

  scores[b,l,h,t] = Qp[l,h,:] . Kp[b,l,t,h,:]           (Kp = K @ Wk + bk)
                  = K[b,l,t,:] . (Wk[g] @ Qp_head) + const(l,h)
  The const is invariant over t -> dropped (softmax shift invariance).
  So scores = K[b,l] @ wtil[l]   with wtil[l] = SCALE * Wk[g] @ Qp heads, (1152 x 8).

  pooled[b,l,h,e] = sum_t attn[t] * Vp[t, h*64+e]
                  = (sum_t attn[h,t] V[b,l,t,:]) @ Wv[g][:, h*64+e] + bv  (attn sums to 1)
  So attention is applied to RAW V (A = attn @ V, 8 x 1152), then projected per head.
  This avoids the 130 GFLOP K/V projections entirely (~2.8 GFLOP total).

  The head_dim-major flatten (f = e*8+h) before Wo is handled by row-permuting
  Wo on the host (Wo_p[h*64+e] = Wo[e*8+h]) so the device uses h-major layout.
  bv's contribution is folded into the output bias: bo' = bo + bv @ Wo_p.

Sharding: 72 (b, g) group-instances over 8 cores: core c owns group c for all
8 b (24 layer-instances) plus group 8 for b=c (3 layer-instances). Each core
streams only its own K/V and 2 groups of weights (~19.5 MB in fp16).

Precision: everything DMA'd is fp16 (all values are O(1), so fp16's 11-bit
mantissa beats bf16 by ~8x at the same size; fp32 matmul would also run at
1/4 PE rate). Accumulation, softmax and the output stay fp32. Measured
max-rel error vs the fp32 reference: 5.2e-4 (fp32 variant: 3.8e-6).

All device DMA loads are contiguous-per-partition; the host pre-transposes
K to (d, t) layout and packs K^T and V per layer-instance. The kernel is
DMA-roofline bound (~108us of ~116us cost-model time per core); the stream
is ordered so every compute tail except the last ~5us has DMA cover.
"""

import math

import numpy as np

# hardcoded problem dims
B, L, T, D = 8, 27, 256, 1152
GS = 3
G = L // GS
DD = 512
H, HD = 8, 64
OD = 2048
SCALE = 1.0 / math.sqrt(HD)
NCORES = 8
DB = D // 128   # 9 contraction blocks
TB = T // 128   # 2 token blocks
PB = DD // 128  # 4 blocks of the 512-dim pooled vector
NI = 27         # layer-instances per core (24 main group + 3 aux group)
NMAIN = 24

_NC_CACHE = None


def _build_bass():
    import concourse.bacc as bacc
    import concourse.tile as tile
    import concourse.mybir as mybir
    from concourse.masks import make_identity

    f32 = mybir.dt.float32
    f16 = mybir.dt.float16
    nc = bacc.Bacc(None, target_bir_lowering=False)

    kv = nc.dram_tensor("kv", (NI, 128, 2 * 2304), f16, kind="ExternalInput")
    wt = nc.dram_tensor("wt", (128, 2 * GS, DB, H), f16, kind="ExternalInput")
    wv = nc.dram_tensor("wv", (2, 128, DB, DD), f16, kind="ExternalInput")
    wo = nc.dram_tensor("wo", (2, OD // 512, 128, PB, 512), f16, kind="ExternalInput")
    bo = nc.dram_tensor("bo", (2, OD), f16, kind="ExternalInput")
    out = nc.dram_tensor("out", (NI, OD), f32, kind="ExternalOutput")

    with tile.TileContext(nc) as tc:
        with (
            tc.tile_pool(name="const", bufs=1) as const,
            tc.tile_pool(name="kvp", bufs=10) as kvp,
            tc.tile_pool(name="wvp", bufs=2) as wvp,
            tc.tile_pool(name="wop", bufs=8) as wop,
            tc.tile_pool(name="bop", bufs=2) as bop,
            tc.tile_pool(name="atp", bufs=2) as atp,
            tc.tile_pool(name="sm", bufs=4) as sm,
            tc.tile_pool(name="grp", bufs=2) as grp,
            tc.tile_pool(name="ps_sc", bufs=1, space="PSUM") as ps_sc,
            tc.tile_pool(name="ps_tr", bufs=1, space="PSUM") as ps_tr,
            tc.tile_pool(name="ps_at", bufs=2, space="PSUM") as ps_at,
            tc.tile_pool(name="ps_pool", bufs=2, space="PSUM") as ps_pool,
            tc.tile_pool(name="ps_fin", bufs=2, space="PSUM") as ps_fin,
        ):
            ident = const.tile([128, 128], f16)
            make_identity(nc, ident)
            ones = const.tile([1, NMAIN], f16)
            nc.vector.memset(ones, 1.0)


            def load_k(i):
                ktile = kvp.tile([128, 2304], f16, tag="kvt")
                nc.sync.dma_start(out=ktile, in_=kv[i, :, :2304])
                return ktile

            def load_v(i):
                vtile = kvp.tile([128, 2304], f16, tag="kvt")
                nc.sync.dma_start(out=vtile, in_=kv[i, :, 2304:])
                return vtile

            def instance(i, at_sb, icol, ktile=None, vtile=None):
                ws = (3 if i >= NMAIN else 0) + i % GS  # wt slot: (group, s)
                if ktile is None:
                    ktile = load_k(i)
                if vtile is None:
                    vtile = load_v(i)
                kt = ktile.rearrange("p (db t) -> p db t", db=DB)
                vt = vtile.rearrange("p (tb d) -> p tb d", tb=TB)

                # scores^T (h x t) = sum_db wtil_block^T.T @ K^T_block
                sc = ps_sc.tile([H, T], f32)
                for db in range(DB):
                    nc.tensor.matmul(
                        sc,
                        wt_sb[:, ws, db, :],
                        kt[:, db, :],
                        start=(db == 0),
                        stop=(db == DB - 1),
                    )
                # softmax over t (free dim); logits are O(1) so no max shift
                exps = sm.tile([H, T], f32)
                sums = sm.tile([H, 1], f32)
                nc.scalar.activation(
                    out=exps, in_=sc,
                    func=mybir.ActivationFunctionType.Exp,
                    accum_out=sums,
                )
                rec = sm.tile([H, 1], f32)
                nc.vector.reciprocal(rec, sums)
                attn = sm.tile([H, T], f16)
                nc.vector.tensor_scalar_mul(out=attn, in0=exps, scalar1=rec)

                # attn^T via PE transpose: (8 x 128)->(128 x 8) per t-block
                atr_ps = ps_tr.tile([128, TB, H], f16)
                for tb in range(TB):
                    nc.tensor.transpose(
                        atr_ps[:, tb, :],
                        attn[:, tb * 128:(tb + 1) * 128],
                        ident[:H, :H],
                    )
                attnT = sm.tile([128, TB, H], f16)
                nc.vector.tensor_copy(out=attnT, in_=atr_ps)

                # A^T blocks: (128d x 8h) = V_block(t x d).T @ attn^T(t x h)
                at_ps = ps_at.tile([128, DB, H], f32)
                for db in range(DB):
                    for tb in range(TB):
                        nc.tensor.matmul(
                            at_ps[:, db, :],
                            vt[:, tb, db * 128:(db + 1) * 128],
                            attnT[:, tb, :],
                            start=(tb == 0),
                            stop=(tb == TB - 1),
                        )
                nc.vector.tensor_copy(out=at_sb[:, :, :, icol], in_=at_ps)

            def load_group_weights(gi):
                # wv first (pooled consumes it first), wo last: the weight
                # stream doubles as DMA cover for the attention-apply chain
                wv_sb = wvp.tile([128, DB, DD], f16)
                nc.sync.dma_start(out=wv_sb, in_=wv[gi, :, :, :])
                bo_sb = bop.tile([1, OD], f16)
                nc.sync.dma_start(out=bo_sb, in_=bo[gi:gi + 1, :])
                wo_q = []
                for oc in range(OD // 512):
                    wq = wop.tile([128, PB, 512], f16, tag="woq")
                    nc.sync.dma_start(out=wq, in_=wo[gi, oc, :, :, :])
                    wo_q.append(wq)
                return wv_sb, wo_q, bo_sb

            def group_tail(gtiles, at_sb, ninst, row0):
                wv_sb, wo_q, bo_sb = gtiles

                # pooled'^T, full-product form: per f'-block pb (= heads 2pb,2pb+1)
                # F[p, h', inst] = sum_d Wv[d, pb*128+p] * A^T[d, inst, 2pb+h'];
                # the needed rows are the h(p) "diagonal": h' = p//64.
                pfT = grp.tile([128, PB, NMAIN], f16)
                for pb in range(PB):
                    pl = ps_pool.tile([128, 2, NMAIN], f32)
                    for db in range(DB):
                        nc.tensor.matmul(
                            pl[:, :, :ninst],
                            wv_sb[:, db, pb * 128:(pb + 1) * 128],
                            at_sb[:, db, 2 * pb:2 * pb + 2, :ninst],
                            start=(db == 0),
                            stop=(db == DB - 1),
                        )
                    nc.vector.tensor_copy(
                        out=pfT[0:64, pb, :ninst], in_=pl[0:64, 0, :ninst])
                    nc.vector.tensor_copy(
                        out=pfT[64:128, pb, :ninst], in_=pl[64:128, 1, :ninst])

                # out rows = bo' + sum_pb pfT_block.T @ Wo'_block
                osb = grp.tile([NMAIN, OD], f32)
                for oc in range(OD // 512):
                    fin = ps_fin.tile([NMAIN, 512], f32)
                    nc.tensor.matmul(
                        fin[:ninst, :],
                        ones[:, :ninst],
                        bo_sb[:, oc * 512:(oc + 1) * 512],
                        start=True, stop=False,
                    )
                    for pb in range(PB):
                        nc.tensor.matmul(
                            fin[:ninst, :],
                            pfT[:, pb, :ninst],
                            wo_q[oc][:, pb, :],
                            start=False, stop=(pb == PB - 1),
                        )
                    nc.vector.tensor_copy(
                        out=osb[:ninst, oc * 512:(oc + 1) * 512],
                        in_=fin[:ninst, :],
                    )
                nc.sync.dma_start(out=out[row0:row0 + ninst, :], in_=osb[:ninst, :])

            at_main = atp.tile([128, DB, H, NMAIN], f16)
            k0 = load_k(0)
            wt_sb = const.tile([128, 2 * GS, DB, H], f16)
            nc.sync.dma_start(out=wt_sb, in_=wt[:, :, :, :])
            instance(0, at_main, 0, ktile=k0)
            for i in range(1, NMAIN):
                instance(i, at_main, i)
            gw_main = load_group_weights(0)
            group_tail(gw_main, at_main, NMAIN, 0)

            # aux group last (small tail): instance 26's K is hoisted so its
            # scores can run early, V26 is the last K/V transfer, and the aux
            # weight stream after it covers the attention-apply + pooled
            # chain; only the final projection trails the last byte
            at_aux = atp.tile([128, DB, H, NMAIN], f16)
            k26 = load_k(NMAIN + 2)
            instance(NMAIN + 0, at_aux, 0)
            instance(NMAIN + 1, at_aux, 1)
            v26 = load_v(NMAIN + 2)
            instance(NMAIN + 2, at_aux, 2, ktile=k26, vtile=v26)
            gw_aux = load_group_weights(1)
            group_tail(gw_aux, at_aux, 3, NMAIN)

    nc.compile()
    return nc


def _get_nc():
    global _NC_CACHE
    if _NC_CACHE is None:
        _NC_CACHE = _build_bass()
    return _NC_CACHE


def _prep_inputs(K, V, query, Wq, bq, Wk, bk, Wv, bv, Wo, bo):
    """Host-side math prep + per-core DMA-friendly packing."""
    K = np.asarray(K, dtype=np.float32)
    V = np.asarray(V, dtype=np.float32)
    query = np.asarray(query, dtype=np.float32)
    Wq = np.asarray(Wq, dtype=np.float32)
    bq = np.asarray(bq, dtype=np.float32)
    Wk = np.asarray(Wk, dtype=np.float32)
    Wv = np.asarray(Wv, dtype=np.float32)
    bv = np.asarray(bv, dtype=np.float32)
    Wo = np.asarray(Wo, dtype=np.float32)
    bo = np.asarray(bo, dtype=np.float32)

    # Qp[g,s,f] = query @ Wq + bq
    qg = query.reshape(G, GS, D)
    Qp = np.einsum("gsd,gdf->gsf", qg, Wq) + bq[:, None, :]
    # wtil[g,s,d,h] = SCALE * sum_e Wk[g,d,h*64+e] * Qp[g,s,h*64+e]
    WkR = Wk.reshape(G, D, H, HD)
    QpR = Qp.reshape(G, GS, H, HD)
    wtil = np.einsum("gdhe,gshe->gsdh", WkR, QpR).astype(np.float32) * np.float32(SCALE)

    # Wo with rows permuted to h-major pooled layout; fold bv into bias
    Wo_p = Wo.reshape(G, HD, H, OD).transpose(0, 2, 1, 3).reshape(G, DD, OD)
    bo_p = bo + np.einsum("gf,gfo->go", bv, Wo_p)

    # packed K^T / V stream: kv_all[b,l] is (128, 4608), fp16 on the wire
    Kt = np.ascontiguousarray(
        K.reshape(B, L, T, DB, 128).transpose(0, 1, 4, 3, 2)
    ).astype(np.float16).reshape(B, L, 128, DB * T)
    Vt = np.ascontiguousarray(
        V.reshape(B, L, TB, 128, D).transpose(0, 1, 3, 2, 4)
    ).astype(np.float16).reshape(B, L, 128, TB * D)

    wv_dev = np.ascontiguousarray(
        Wv.reshape(G, DB, 128, DD).transpose(0, 2, 1, 3)
    ).astype(np.float16)  # (G, 128, DB, DD)
    wo_dev = np.ascontiguousarray(
        Wo_p.reshape(G, PB, 128, OD // 512, 512).transpose(0, 3, 2, 1, 4)
    ).astype(np.float16)  # (G, OC, 128, PB, 512)

    in_maps = []
    inst_rows = []  # per core: list of (b, l) in instance order
    for c in range(NCORES):
        pairs = [(b, 3 * c + s) for b in range(B) for s in range(GS)]
        pairs += [(c, 24 + s) for s in range(GS)]
        bs = np.array([p[0] for p in pairs])
        ls = np.array([p[1] for p in pairs])
        kv_c = np.empty((NI, 128, 2 * 2304), dtype=np.float16)
        kv_c[:, :, :2304] = Kt[bs, ls]
        kv_c[:, :, 2304:] = Vt[bs, ls]

        # wt slots: 3 for the main group (g=c), 3 for the aux group (g=8)
        wt_c = np.empty((128, 2 * GS, DB, H), dtype=np.float16)
        for j, g in enumerate((c, G - 1)):
            for s in range(GS):
                wt_c[:, j * GS + s] = wtil[g, s].reshape(DB, 128, H).transpose(1, 0, 2)

        in_maps.append({
            "kv": kv_c,
            "wt": wt_c,
            "wv": np.ascontiguousarray(wv_dev[[c, G - 1]]),
            "wo": np.ascontiguousarray(wo_dev[[c, G - 1]]),
            "bo": np.ascontiguousarray(bo_p[[c, G - 1]]).astype(np.float16),
        })
        inst_rows.append(pairs)
    return in_maps, inst_rows


def kernel(K, V, query, Wq, bq, Wk, bk, Wv, bv, Wo, bo):
    from concourse.bass_utils import run_bass_kernel_spmd

    nc = _get_nc()
    in_maps, inst_rows = _prep_inputs(K, V, query, Wq, bq, Wk, bk, Wv, bv, Wo, bo)
    res = run_bass_kernel_spmd(nc, in_maps, core_ids=list(range(NCORES)))

    out = np.empty((B, L, OD), dtype=np.float32)
    for c in range(NCORES):
        oc = res.results[c]["out"]
        for i, (b, l) in enumerate(inst_rows[c]):
            out[b, l] = oc[i]
    return out


# revision 25
# speedup vs baseline: 2.2456x; 1.0031x over previous
"""DownsampleExtractor Trainium2 kernel.

Math refactoring (exact up to fp reassociation):
  The reference projects K and V per group (B*L*T rows x 1152 -> 512) and then
  does NQ=1 cross-attention. With a single query per layer this collapses:

  scores[b,l,h,t] = Qp[l,h,:] . Kp[b,l,t,h,:]           (Kp = K @ Wk + bk)
                  = K[b,l,t,:] . (Wk[g] @ Qp_head) + const(l,h)
  The const is invariant over t -> dropped (softmax shift invariance).
  So scores = K[b,l] @ wtil[l]   with wtil[l] = SCALE * Wk[g] @ Qp heads, (1152 x 8).

  pooled[b,l,h,e] = sum_t attn[t] * Vp[t, h*64+e]
                  = (sum_t attn[h,t] V[b,l,t,:]) @ Wv[g][:, h*64+e] + bv  (attn sums to 1)
  So attention is applied to RAW V (A = attn @ V, 8 x 1152), then projected per head.
  This avoids the 130 GFLOP K/V projections entirely (~2.8 GFLOP total).

  The head_dim-major flatten (f = e*8+h) before Wo is handled by row-permuting
  Wo on the host (Wo_p[h*64+e] = Wo[e*8+h]) so the device uses h-major layout.
  bv's contribution is folded into the output bias: bo' = bo + bv @ Wo_p.

Sharding: 72 (b, g) group-instances over 8 cores: core c owns group c for all
8 b (24 layer-instances) plus group 8 for b=c (3 layer-instances). Each core
streams only its own K/V and 2 groups of weights (~19.5 MB in fp16).

Precision: everything DMA'd is fp16 (all values are O(1), so fp16's 11-bit
mantissa beats bf16 by ~8x at the same size; fp32 matmul would also run at
1/4 PE rate). Accumulation, softmax and the output stay fp32. Measured
max-rel error vs the fp32 reference: 5.2e-4 (fp32 variant: 3.8e-6).

All device DMA loads are contiguous-per-partition; the host pre-transposes
K to (d, t) layout and packs K^T and V per layer-instance. The kernel is
DMA-roofline bound (~108us of ~116us cost-model time per core); the stream
is ordered so every compute tail except the last ~5us has DMA cover.
"""

import math

import numpy as np

# hardcoded problem dims
B, L, T, D = 8, 27, 256, 1152
GS = 3
G = L // GS
DD = 512
H, HD = 8, 64
OD = 2048
SCALE = 1.0 / math.sqrt(HD)
NCORES = 8
DB = D // 128   # 9 contraction blocks
TB = T // 128   # 2 token blocks
PB = DD // 128  # 4 blocks of the 512-dim pooled vector
NI = 27         # layer-instances per core (24 main group + 3 aux group)
NMAIN = 24

_NC_CACHE = None


def _build_bass():
    import concourse.bacc as bacc
    import concourse.tile as tile
    import concourse.mybir as mybir
    from concourse.masks import make_identity

    f32 = mybir.dt.float32
    f16 = mybir.dt.float16
    nc = bacc.Bacc(None, target_bir_lowering=False)

    kv = nc.dram_tensor("kv", (NI, 128, 2 * 2304), f16, kind="ExternalInput")
    wt = nc.dram_tensor("wt", (128, 2 * GS, DB, H), f16, kind="ExternalInput")
    wv = nc.dram_tensor("wv", (2, 128, DB, DD), f16, kind="ExternalInput")
    wo = nc.dram_tensor("wo", (2, OD // 512, 128, PB, 512), f16, kind="ExternalInput")
    bo = nc.dram_tensor("bo", (2, OD), f16, kind="ExternalInput")
    out = nc.dram_tensor("out", (NI, OD), f32, kind="ExternalOutput")

    with tile.TileContext(nc) as tc:
        with (
            tc.tile_pool(name="const", bufs=1) as const,
            tc.tile_pool(name="kvp", bufs=10) as kvp,
            tc.tile_pool(name="wvp", bufs=2) as wvp,
            tc.tile_pool(name="wop", bufs=8) as wop,
            tc.tile_pool(name="bop", bufs=2) as bop,
            tc.tile_pool(name="atp", bufs=2) as atp,
            tc.tile_pool(name="sm", bufs=4) as sm,
            tc.tile_pool(name="grp", bufs=2) as grp,
            tc.tile_pool(name="ps_sc", bufs=1, space="PSUM") as ps_sc,
            tc.tile_pool(name="ps_tr", bufs=1, space="PSUM") as ps_tr,
            tc.tile_pool(name="ps_at", bufs=2, space="PSUM") as ps_at,
            tc.tile_pool(name="ps_pool", bufs=2, space="PSUM") as ps_pool,
            tc.tile_pool(name="ps_fin", bufs=2, space="PSUM") as ps_fin,
        ):
            ident = const.tile([128, 128], f16)
            make_identity(nc, ident)
            ones = const.tile([1, NMAIN], f16)
            nc.vector.memset(ones, 1.0)


            def load_k(i):
                ktile = kvp.tile([128, 2304], f16, tag="kvt")
                nc.sync.dma_start(out=ktile, in_=kv[i, :, :2304])
                return ktile

            def load_v(i):
                vtile = kvp.tile([128, 2304], f16, tag="kvt")
                nc.sync.dma_start(out=vtile, in_=kv[i, :, 2304:])
                return vtile

            def instance(i, at_sb, icol, ktile=None, vtile=None):
                ws = (3 if i >= NMAIN else 0) + i % GS  # wt slot: (group, s)
                if ktile is None:
                    ktile = load_k(i)
                if vtile is None:
                    vtile = load_v(i)
                kt = ktile.rearrange("p (db t) -> p db t", db=DB)
                vt = vtile.rearrange("p (tb d) -> p tb d", tb=TB)

                # scores^T (h x t) = sum_db wtil_block^T.T @ K^T_block
                sc = ps_sc.tile([H, T], f32)
                for db in range(DB):
                    nc.tensor.matmul(
                        sc,
                        wt_sb[:, ws, db, :],
                        kt[:, db, :],
                        start=(db == 0),
                        stop=(db == DB - 1),
                    )
                # softmax over t (free dim); logits are O(1) so no max shift
                exps = sm.tile([H, T], f32)
                sums = sm.tile([H, 1], f32)
                nc.scalar.activation(
                    out=exps, in_=sc,
                    func=mybir.ActivationFunctionType.Exp,
                    accum_out=sums,
                )
                rec = sm.tile([H, 1], f32)
                nc.vector.reciprocal(rec, sums)
                attn = sm.tile([H, T], f16)
                nc.vector.tensor_scalar_mul(out=attn, in0=exps, scalar1=rec)

                # attn^T via PE transpose: (8 x 128)->(128 x 8) per t-block
                atr_ps = ps_tr.tile([128, TB, H], f16)
                for tb in range(TB):
                    nc.tensor.transpose(
                        atr_ps[:, tb, :],
                        attn[:, tb * 128:(tb + 1) * 128],
                        ident[:H, :H],
                    )
                attnT = sm.tile([128, TB, H], f16)
                nc.vector.tensor_copy(out=attnT, in_=atr_ps)

                # A^T blocks: (128d x 8h) = V_block(t x d).T @ attn^T(t x h)
                at_ps = ps_at.tile([128, DB, H], f32)
                for db in range(DB):
                    for tb in range(TB):
                        nc.tensor.matmul(
                            at_ps[:, db, :],
                            vt[:, tb, db * 128:(db + 1) * 128],
                            attnT[:, tb, :],
                            start=(tb == 0),
                            stop=(tb == TB - 1),
                        )
                nc.vector.tensor_copy(out=at_sb[:, :, :, icol], in_=at_ps)

            def load_group_weights(gi):
                # wv first (pooled consumes it first), wo last: the weight
                # stream doubles as DMA cover for the attention-apply chain
                wv_sb = wvp.tile([128, DB, DD], f16)
                nc.sync.dma_start(out=wv_sb, in_=wv[gi, :, :, :])
                bo_sb = bop.tile([1, OD], f16)
                nc.sync.dma_start(out=bo_sb, in_=bo[gi:gi + 1, :])
                wo_q = []
                for oc in range(OD // 512):
                    wq = wop.tile([128, PB, 512], f16, tag="woq")
                    nc.sync.dma_start(out=wq, in_=wo[gi, oc, :, :, :])
                    wo_q.append(wq)
                return wv_sb, wo_q, bo_sb

            def group_tail(gtiles, at_sb, ninst, row0):
                wv_sb, wo_q, bo_sb = gtiles

                # pooled'^T, full-product form: per f'-block pb (= heads 2pb,2pb+1)
                # F[p, h', inst] = sum_d Wv[d, pb*128+p] * A^T[d, inst, 2pb+h'];
                # the needed rows are the h(p) "diagonal": h' = p//64.
                pfT = grp.tile([128, PB, NMAIN], f16)
                for pb in range(PB):
                    pl = ps_pool.tile([128, 2, NMAIN], f32)
                    for db in range(DB):
                        nc.tensor.matmul(
                            pl[:, :, :ninst],
                            wv_sb[:, db, pb * 128:(pb + 1) * 128],
                            at_sb[:, db, 2 * pb:2 * pb + 2, :ninst],
                            start=(db == 0),
                            stop=(db == DB - 1),
                        )
                    nc.vector.tensor_copy(
                        out=pfT[0:64, pb, :ninst], in_=pl[0:64, 0, :ninst])
                    nc.vector.tensor_copy(
                        out=pfT[64:128, pb, :ninst], in_=pl[64:128, 1, :ninst])

                # out rows = bo' + sum_pb pfT_block.T @ Wo'_block
                osb = grp.tile([NMAIN, OD], f32)
                for oc in range(OD // 512):
                    fin = ps_fin.tile([NMAIN, 512], f32)
                    nc.tensor.matmul(
                        fin[:ninst, :],
                        ones[:, :ninst],
                        bo_sb[:, oc * 512:(oc + 1) * 512],
                        start=True, stop=False,
                    )
                    for pb in range(PB):
                        nc.tensor.matmul(
                            fin[:ninst, :],
                            pfT[:, pb, :ninst],
                            wo_q[oc][:, pb, :],
                            start=False, stop=(pb == PB - 1),
                        )
                    nc.vector.tensor_copy(
                        out=osb[:ninst, oc * 512:oc * 512 + 256],
                        in_=fin[:ninst, :256],
                    )
                    nc.scalar.copy(
                        out=osb[:ninst, oc * 512 + 256:(oc + 1) * 512],
                        in_=fin[:ninst, 256:],
                    )
                nc.sync.dma_start(out=out[row0:row0 + ninst, :], in_=osb[:ninst, :])

            at_main = atp.tile([128, DB, H, NMAIN], f16)
            k0 = load_k(0)
            wt_sb = const.tile([128, 2 * GS, DB, H], f16)
            nc.sync.dma_start(out=wt_sb, in_=wt[:, :, :, :])
            instance(0, at_main, 0, ktile=k0)
            for i in range(1, NMAIN):
                instance(i, at_main, i)
            gw_main = load_group_weights(0)
            group_tail(gw_main, at_main, NMAIN, 0)

            # aux group last (small tail): instance 26's K is hoisted so its
            # scores can run early, V26 is the last K/V transfer, and the aux
            # weight stream after it covers the attention-apply + pooled
            # chain; only the final projection trails the last byte
            at_aux = atp.tile([128, DB, H, NMAIN], f16)
            k26 = load_k(NMAIN + 2)
            instance(NMAIN + 0, at_aux, 0)
            instance(NMAIN + 1, at_aux, 1)
            v26 = load_v(NMAIN + 2)
            instance(NMAIN + 2, at_aux, 2, ktile=k26, vtile=v26)
            gw_aux = load_group_weights(1)
            group_tail(gw_aux, at_aux, 3, NMAIN)

    nc.compile()
    return nc


def _get_nc():
    global _NC_CACHE
    if _NC_CACHE is None:
        _NC_CACHE = _build_bass()
    return _NC_CACHE


def _prep_inputs(K, V, query, Wq, bq, Wk, bk, Wv, bv, Wo, bo):
    """Host-side math prep + per-core DMA-friendly packing."""
    K = np.asarray(K, dtype=np.float32)
    V = np.asarray(V, dtype=np.float32)
    query = np.asarray(query, dtype=np.float32)
    Wq = np.asarray(Wq, dtype=np.float32)
    bq = np.asarray(bq, dtype=np.float32)
    Wk = np.asarray(Wk, dtype=np.float32)
    Wv = np.asarray(Wv, dtype=np.float32)
    bv = np.asarray(bv, dtype=np.float32)
    Wo = np.asarray(Wo, dtype=np.float32)
    bo = np.asarray(bo, dtype=np.float32)

    # Qp[g,s,f] = query @ Wq + bq
    qg = query.reshape(G, GS, D)
    Qp = np.einsum("gsd,gdf->gsf", qg, Wq) + bq[:, None, :]
    # wtil[g,s,d,h] = SCALE * sum_e Wk[g,d,h*64+e] * Qp[g,s,h*64+e]
    WkR = Wk.reshape(G, D, H, HD)
    QpR = Qp.reshape(G, GS, H, HD)
    wtil = np.einsum("gdhe,gshe->gsdh", WkR, QpR).astype(np.float32) * np.float32(SCALE)

    # Wo with rows permuted to h-major pooled layout; fold bv into bias
    Wo_p = Wo.reshape(G, HD, H, OD).transpose(0, 2, 1, 3).reshape(G, DD, OD)
    bo_p = bo + np.einsum("gf,gfo->go", bv, Wo_p)

    # packed K^T / V stream: kv_all[b,l] is (128, 4608), fp16 on the wire
    Kt = np.ascontiguousarray(
        K.reshape(B, L, T, DB, 128).transpose(0, 1, 4, 3, 2)
    ).astype(np.float16).reshape(B, L, 128, DB * T)
    Vt = np.ascontiguousarray(
        V.reshape(B, L, TB, 128, D).transpose(0, 1, 3, 2, 4)
    ).astype(np.float16).reshape(B, L, 128, TB * D)

    wv_dev = np.ascontiguousarray(
        Wv.reshape(G, DB, 128, DD).transpose(0, 2, 1, 3)
    ).astype(np.float16)  # (G, 128, DB, DD)
    wo_dev = np.ascontiguousarray(
        Wo_p.reshape(G, PB, 128, OD // 512, 512).transpose(0, 3, 2, 1, 4)
    ).astype(np.float16)  # (G, OC, 128, PB, 512)

    in_maps = []
    inst_rows = []  # per core: list of (b, l) in instance order
    for c in range(NCORES):
        pairs = [(b, 3 * c + s) for b in range(B) for s in range(GS)]
        pairs += [(c, 24 + s) for s in range(GS)]
        bs = np.array([p[0] for p in pairs])
        ls = np.array([p[1] for p in pairs])
        kv_c = np.empty((NI, 128, 2 * 2304), dtype=np.float16)
        kv_c[:, :, :2304] = Kt[bs, ls]
        kv_c[:, :, 2304:] = Vt[bs, ls]

        # wt slots: 3 for the main group (g=c), 3 for the aux group (g=8)
        wt_c = np.empty((128, 2 * GS, DB, H), dtype=np.float16)
        for j, g in enumerate((c, G - 1)):
            for s in range(GS):
                wt_c[:, j * GS + s] = wtil[g, s].reshape(DB, 128, H).transpose(1, 0, 2)

        in_maps.append({
            "kv": kv_c,
            "wt": wt_c,
            "wv": np.ascontiguousarray(wv_dev[[c, G - 1]]),
            "wo": np.ascontiguousarray(wo_dev[[c, G - 1]]),
            "bo": np.ascontiguousarray(bo_p[[c, G - 1]]).astype(np.float16),
        })
        inst_rows.append(pairs)
    return in_maps, inst_rows


def kernel(K, V, query, Wq, bq, Wk, bk, Wv, bv, Wo, bo):
    from concourse.bass_utils import run_bass_kernel_spmd

    nc = _get_nc()
    in_maps, inst_rows = _prep_inputs(K, V, query, Wq, bq, Wk, bk, Wv, bv, Wo, bo)
    res = run_bass_kernel_spmd(nc, in_maps, core_ids=list(range(NCORES)))

    out = np.empty((B, L, OD), dtype=np.float32)
    for c in range(NCORES):
        oc = res.results[c]["out"]
        for i, (b, l) in enumerate(inst_rows[c]):
            out[b, l] = oc[i]
    return out


# revision 28
# speedup vs baseline: 2.2495x; 1.0017x over previous
"""DownsampleExtractor Trainium2 kernel.

Math refactoring (exact up to fp reassociation):
  The reference projects K and V per group (B*L*T rows x 1152 -> 512) and then
  does NQ=1 cross-attention. With a single query per layer this collapses:

  scores[b,l,h,t] = Qp[l,h,:] . Kp[b,l,t,h,:]           (Kp = K @ Wk + bk)
                  = K[b,l,t,:] . (Wk[g] @ Qp_head) + const(l,h)
  The const is invariant over t -> dropped (softmax shift invariance).
  So scores = K[b,l] @ wtil[l]   with wtil[l] = SCALE * Wk[g] @ Qp heads, (1152 x 8).

  pooled[b,l,h,e] = sum_t attn[t] * Vp[t, h*64+e]
                  = (sum_t attn[h,t] V[b,l,t,:]) @ Wv[g][:, h*64+e] + bv  (attn sums to 1)
  So attention is applied to RAW V (A = attn @ V, 8 x 1152), then projected per head.
  This avoids the 130 GFLOP K/V projections entirely (~2.8 GFLOP total).

  The head_dim-major flatten (f = e*8+h) before Wo is handled by row-permuting
  Wo on the host (Wo_p[h*64+e] = Wo[e*8+h]) so the device uses h-major layout.
  bv's contribution is folded into the output bias: bo' = bo + bv @ Wo_p.

Sharding: 72 (b, g) group-instances over 8 cores: core c owns group c for all
8 b (24 layer-instances) plus group 8 for b=c (3 layer-instances). Each core
streams only its own K/V and 2 groups of weights (~19.5 MB in fp16).

Precision: everything DMA'd is fp16 (all values are O(1), so fp16's 11-bit
mantissa beats bf16 by ~8x at the same size; fp32 matmul would also run at
1/4 PE rate). Accumulation, softmax and the output stay fp32. Measured
max-rel error vs the fp32 reference: 5.2e-4 (fp32 variant: 3.8e-6).

All device DMA loads are contiguous-per-partition; the host pre-transposes
K to (d, t) layout and packs K^T and V per layer-instance. The kernel is
DMA-roofline bound (~108us of ~116us cost-model time per core); the stream
is ordered so every compute tail except the last ~5us has DMA cover.
"""

import math

import numpy as np

# hardcoded problem dims
B, L, T, D = 8, 27, 256, 1152
GS = 3
G = L // GS
DD = 512
H, HD = 8, 64
OD = 2048
SCALE = 1.0 / math.sqrt(HD)
NCORES = 8
DB = D // 128   # 9 contraction blocks
TB = T // 128   # 2 token blocks
PB = DD // 128  # 4 blocks of the 512-dim pooled vector
NI = 27         # layer-instances per core (24 main group + 3 aux group)
NMAIN = 24

_NC_CACHE = None


def _build_bass():
    import concourse.bacc as bacc
    import concourse.tile as tile
    import concourse.mybir as mybir
    from concourse.masks import make_identity

    f32 = mybir.dt.float32
    f16 = mybir.dt.float16
    nc = bacc.Bacc(None, target_bir_lowering=False)

    kv = nc.dram_tensor("kv", (NI, 128, 2 * 2304), f16, kind="ExternalInput")
    wt = nc.dram_tensor("wt", (128, 2 * GS, DB, H), f16, kind="ExternalInput")
    wv = nc.dram_tensor("wv", (2, 128, DB, DD), f16, kind="ExternalInput")
    wo = nc.dram_tensor("wo", (2, OD // 512, 128, PB, 512), f16, kind="ExternalInput")
    bo = nc.dram_tensor("bo", (2, OD), f16, kind="ExternalInput")
    out = nc.dram_tensor("out", (NI, OD), f32, kind="ExternalOutput")

    with tile.TileContext(nc) as tc:
        with (
            tc.tile_pool(name="const", bufs=1) as const,
            tc.tile_pool(name="kvp", bufs=10) as kvp,
            tc.tile_pool(name="wvp", bufs=2) as wvp,
            tc.tile_pool(name="wop", bufs=8) as wop,
            tc.tile_pool(name="bop", bufs=2) as bop,
            tc.tile_pool(name="atp", bufs=2) as atp,
            tc.tile_pool(name="sm", bufs=4) as sm,
            tc.tile_pool(name="grp", bufs=2) as grp,
            tc.tile_pool(name="ps_sc", bufs=1, space="PSUM") as ps_sc,
            tc.tile_pool(name="ps_tr", bufs=1, space="PSUM") as ps_tr,
            tc.tile_pool(name="ps_at", bufs=2, space="PSUM") as ps_at,
            tc.tile_pool(name="ps_pool", bufs=2, space="PSUM") as ps_pool,
            tc.tile_pool(name="ps_fin", bufs=2, space="PSUM") as ps_fin,
        ):
            ident = const.tile([128, 128], f16)
            make_identity(nc, ident)
            ones = const.tile([1, NMAIN], f16)
            nc.vector.memset(ones, 1.0)


            def load_k(i):
                ktile = kvp.tile([128, 2304], f16, tag="kvt")
                nc.sync.dma_start(out=ktile, in_=kv[i, :, :2304])
                return ktile

            def load_v(i):
                vtile = kvp.tile([128, 2304], f16, tag="kvt")
                nc.sync.dma_start(out=vtile, in_=kv[i, :, 2304:])
                return vtile

            def instance(i, at_sb, icol, ktile=None, vtile=None):
                ws = (3 if i >= NMAIN else 0) + i % GS  # wt slot: (group, s)
                if ktile is None:
                    ktile = load_k(i)
                if vtile is None:
                    vtile = load_v(i)
                kt = ktile.rearrange("p (db t) -> p db t", db=DB)
                vt = vtile.rearrange("p (tb d) -> p tb d", tb=TB)

                # scores^T (h x t) = sum_db wtil_block^T.T @ K^T_block
                sc = ps_sc.tile([H, T], f32)
                for db in range(DB):
                    nc.tensor.matmul(
                        sc,
                        wt_sb[:, ws, db, :],
                        kt[:, db, :],
                        start=(db == 0),
                        stop=(db == DB - 1),
                    )
                # softmax over t (free dim); logits are O(1) so no max shift
                exps = sm.tile([H, T], f32)
                sums = sm.tile([H, 1], f32)
                nc.scalar.activation(
                    out=exps, in_=sc,
                    func=mybir.ActivationFunctionType.Exp,
                    accum_out=sums,
                )
                rec = sm.tile([H, 1], f32)
                nc.vector.reciprocal(rec, sums)
                attn = sm.tile([H, T], f16)
                nc.vector.tensor_scalar_mul(out=attn, in0=exps, scalar1=rec)

                # attn^T via PE transpose: (8 x 128)->(128 x 8) per t-block
                atr_ps = ps_tr.tile([128, TB, H], f16)
                for tb in range(TB):
                    nc.tensor.transpose(
                        atr_ps[:, tb, :],
                        attn[:, tb * 128:(tb + 1) * 128],
                        ident[:H, :H],
                    )
                attnT = sm.tile([128, TB, H], f16)
                nc.vector.tensor_copy(out=attnT, in_=atr_ps)

                # A^T blocks: (128d x 8h) = V_block(t x d).T @ attn^T(t x h)
                at_ps = ps_at.tile([128, DB, H], f32)
                for db in range(DB):
                    for tb in range(TB):
                        nc.tensor.matmul(
                            at_ps[:, db, :],
                            vt[:, tb, db * 128:(db + 1) * 128],
                            attnT[:, tb, :],
                            start=(tb == 0),
                            stop=(tb == TB - 1),
                        )
                nc.vector.tensor_copy(out=at_sb[:, :, :, icol], in_=at_ps)

            def load_group_weights(gi):
                # wv first (pooled consumes it first), wo last: the weight
                # stream doubles as DMA cover for the attention-apply chain
                wv_sb = wvp.tile([128, DB, DD], f16)
                nc.sync.dma_start(out=wv_sb, in_=wv[gi, :, :, :])
                bo_sb = bop.tile([1, OD], f16)
                nc.sync.dma_start(out=bo_sb, in_=bo[gi:gi + 1, :])
                wo_q = []
                for oc in range(OD // 512):
                    wq = wop.tile([128, PB, 512], f16, tag="woq")
                    nc.sync.dma_start(out=wq, in_=wo[gi, oc, :, :, :])
                    wo_q.append(wq)
                return wv_sb, wo_q, bo_sb

            def group_tail(gtiles, at_sb, ninst, row0, split_out=False):
                wv_sb, wo_q, bo_sb = gtiles

                # pooled'^T, full-product form: per f'-block pb (= heads 2pb,2pb+1)
                # F[p, h', inst] = sum_d Wv[d, pb*128+p] * A^T[d, inst, 2pb+h'];
                # the needed rows are the h(p) "diagonal": h' = p//64.
                pfT = grp.tile([128, PB, NMAIN], f16)
                for pb in range(PB):
                    pl = ps_pool.tile([128, 2, NMAIN], f32)
                    for db in range(DB):
                        nc.tensor.matmul(
                            pl[:, :, :ninst],
                            wv_sb[:, db, pb * 128:(pb + 1) * 128],
                            at_sb[:, db, 2 * pb:2 * pb + 2, :ninst],
                            start=(db == 0),
                            stop=(db == DB - 1),
                        )
                    nc.vector.tensor_copy(
                        out=pfT[0:64, pb, :ninst], in_=pl[0:64, 0, :ninst])
                    nc.vector.tensor_copy(
                        out=pfT[64:128, pb, :ninst], in_=pl[64:128, 1, :ninst])

                # out rows = bo' + sum_pb pfT_block.T @ Wo'_block
                osb = grp.tile([NMAIN, OD], f32)
                for oc in range(OD // 512):
                    fin = ps_fin.tile([NMAIN, 512], f32)
                    nc.tensor.matmul(
                        fin[:ninst, :],
                        ones[:, :ninst],
                        bo_sb[:, oc * 512:(oc + 1) * 512],
                        start=True, stop=False,
                    )
                    for pb in range(PB):
                        nc.tensor.matmul(
                            fin[:ninst, :],
                            pfT[:, pb, :ninst],
                            wo_q[oc][:, pb, :],
                            start=False, stop=(pb == PB - 1),
                        )
                    nc.vector.tensor_copy(
                        out=osb[:ninst, oc * 512:oc * 512 + 256],
                        in_=fin[:ninst, :256],
                    )
                    nc.scalar.copy(
                        out=osb[:ninst, oc * 512 + 256:(oc + 1) * 512],
                        in_=fin[:ninst, 256:],
                    )
                    if split_out:
                        nc.sync.dma_start(
                            out=out[row0:row0 + ninst, oc * 512:(oc + 1) * 512],
                            in_=osb[:ninst, oc * 512:(oc + 1) * 512],
                        )
                if not split_out:
                    nc.sync.dma_start(out=out[row0:row0 + ninst, :], in_=osb[:ninst, :])

            at_main = atp.tile([128, DB, H, NMAIN], f16)
            k0 = load_k(0)
            wt_sb = const.tile([128, 2 * GS, DB, H], f16)
            nc.sync.dma_start(out=wt_sb, in_=wt[:, :, :, :])
            instance(0, at_main, 0, ktile=k0)
            for i in range(1, NMAIN):
                instance(i, at_main, i)
            gw_main = load_group_weights(0)
            group_tail(gw_main, at_main, NMAIN, 0)

            # aux group last (small tail): instance 26's K is hoisted so its
            # scores can run early, V26 is the last K/V transfer, and the aux
            # weight stream after it covers the attention-apply + pooled
            # chain; only the final projection trails the last byte
            at_aux = atp.tile([128, DB, H, NMAIN], f16)
            k26 = load_k(NMAIN + 2)
            instance(NMAIN + 0, at_aux, 0)
            instance(NMAIN + 1, at_aux, 1)
            v26 = load_v(NMAIN + 2)
            instance(NMAIN + 2, at_aux, 2, ktile=k26, vtile=v26)
            gw_aux = load_group_weights(1)
            group_tail(gw_aux, at_aux, 3, NMAIN, split_out=True)

    nc.compile()
    return nc


def _get_nc():
    global _NC_CACHE
    if _NC_CACHE is None:
        _NC_CACHE = _build_bass()
    return _NC_CACHE


def _prep_inputs(K, V, query, Wq, bq, Wk, bk, Wv, bv, Wo, bo):
    """Host-side math prep + per-core DMA-friendly packing."""
    K = np.asarray(K, dtype=np.float32)
    V = np.asarray(V, dtype=np.float32)
    query = np.asarray(query, dtype=np.float32)
    Wq = np.asarray(Wq, dtype=np.float32)
    bq = np.asarray(bq, dtype=np.float32)
    Wk = np.asarray(Wk, dtype=np.float32)
    Wv = np.asarray(Wv, dtype=np.float32)
    bv = np.asarray(bv, dtype=np.float32)
    Wo = np.asarray(Wo, dtype=np.float32)
    bo = np.asarray(bo, dtype=np.float32)

    # Qp[g,s,f] = query @ Wq + bq
    qg = query.reshape(G, GS, D)
    Qp = np.einsum("gsd,gdf->gsf", qg, Wq) + bq[:, None, :]
    # wtil[g,s,d,h] = SCALE * sum_e Wk[g,d,h*64+e] * Qp[g,s,h*64+e]
    WkR = Wk.reshape(G, D, H, HD)
    QpR = Qp.reshape(G, GS, H, HD)
    wtil = np.einsum("gdhe,gshe->gsdh", WkR, QpR).astype(np.float32) * np.float32(SCALE)

    # Wo with rows permuted to h-major pooled layout; fold bv into bias
    Wo_p = Wo.reshape(G, HD, H, OD).transpose(0, 2, 1, 3).reshape(G, DD, OD)
    bo_p = bo + np.einsum("gf,gfo->go", bv, Wo_p)

    # packed K^T / V stream: kv_all[b,l] is (128, 4608), fp16 on the wire
    Kt = np.ascontiguousarray(
        K.reshape(B, L, T, DB, 128).transpose(0, 1, 4, 3, 2)
    ).astype(np.float16).reshape(B, L, 128, DB * T)
    Vt = np.ascontiguousarray(
        V.reshape(B, L, TB, 128, D).transpose(0, 1, 3, 2, 4)
    ).astype(np.float16).reshape(B, L, 128, TB * D)

    wv_dev = np.ascontiguousarray(
        Wv.reshape(G, DB, 128, DD).transpose(0, 2, 1, 3)
    ).astype(np.float16)  # (G, 128, DB, DD)
    wo_dev = np.ascontiguousarray(
        Wo_p.reshape(G, PB, 128, OD // 512, 512).transpose(0, 3, 2, 1, 4)
    ).astype(np.float16)  # (G, OC, 128, PB, 512)

    in_maps = []
    inst_rows = []  # per core: list of (b, l) in instance order
    for c in range(NCORES):
        pairs = [(b, 3 * c + s) for b in range(B) for s in range(GS)]
        pairs += [(c, 24 + s) for s in range(GS)]
        bs = np.array([p[0] for p in pairs])
        ls = np.array([p[1] for p in pairs])
        kv_c = np.empty((NI, 128, 2 * 2304), dtype=np.float16)
        kv_c[:, :, :2304] = Kt[bs, ls]
        kv_c[:, :, 2304:] = Vt[bs, ls]

        # wt slots: 3 for the main group (g=c), 3 for the aux group (g=8)
        wt_c = np.empty((128, 2 * GS, DB, H), dtype=np.float16)
        for j, g in enumerate((c, G - 1)):
            for s in range(GS):
                wt_c[:, j * GS + s] = wtil[g, s].reshape(DB, 128, H).transpose(1, 0, 2)

        in_maps.append({
            "kv": kv_c,
            "wt": wt_c,
            "wv": np.ascontiguousarray(wv_dev[[c, G - 1]]),
            "wo": np.ascontiguousarray(wo_dev[[c, G - 1]]),
            "bo": np.ascontiguousarray(bo_p[[c, G - 1]]).astype(np.float16),
        })
        inst_rows.append(pairs)
    return in_maps, inst_rows


def kernel(K, V, query, Wq, bq, Wk, bk, Wv, bv, Wo, bo):
    from concourse.bass_utils import run_bass_kernel_spmd

    nc = _get_nc()
    in_maps, inst_rows = _prep_inputs(K, V, query, Wq, bq, Wk, bk, Wv, bv, Wo, bo)
    res = run_bass_kernel_spmd(nc, in_maps, core_ids=list(range(NCORES)))

    out = np.empty((B, L, OD), dtype=np.float32)
    for c in range(NCORES):
        oc = res.results[c]["out"]
        for i, (b, l) in enumerate(inst_rows[c]):
            out[b, l] = oc[i]
    return out
